# revision 32
# baseline (speedup 1.0000x reference)
"""Feedforward SNN (Linear -> LIF) x2 kernel for Trainium2, 8-core data parallel.

Per-core plan (B sharded 8 ways, BL=32 samples/core):
  - Host pre-transposes operands once (cheap numpy) and Dekker-splits EVERY
    matmul operand into fp16 hi+lo pairs (pre-scaled by powers of 2 to dodge
    fp16 subnormals). fp16 runs the PE at 1.0 cycles/row (vs fp32's 4.0) and
    its 11-bit mantissa is held exactly by the PE's internal FP22 format, so
    a 2-term split carries ~22 significant bits -- fp32-grade for this model
    (validated: end-to-end error below the fp32 BLAS-reorder noise floor).
  - Layer-1 currents for ALL timesteps: Cur1[h1, (t,b)] = W1 @ x^T as THREE
    fp16 matmuls (W1h*xh + W1l*xh + W1h*xl; the dropped W1l*xl term is
    ~2^-22 relative). 3 passes at 1x rate vs fp32's 4x-slow single pass.
  - LIF-1 scan over t on [128, HC1*32] tiles (partition = h1 % 128, free =
    (h1chunk, b)); fused scalar_tensor_tensor DVE ops, 3/step.
  - Spikes are {0,1} == exact in fp16; layer-2 currents are 2x fp16 matmuls
    (W2h + W2l = 22-bit W2) accumulated in fp32 PSUM.
  - The pre-scales are undone for free in the PSUM->SBUF bias-add
    activation (out = psum*scale + bias).
  - LIF-2 scan likewise (2 DVE ops/step; spikes materialized only at t=63).
  - Software-pipelined: mm1(nb+1) is emitted before mm2(nb) so the PE fills
    the scan1(nb) latency; PE phases are chained with order-only deps.
"""

import os
import sys

import numpy as np

for _p in ("/opt/trn_rl_repo", "/root/.axon_site/_ro/trn_rl_repo"):
    if os.path.isdir(_p) and _p not in sys.path:
        sys.path.insert(0, _p)

import ml_dtypes  # noqa: E402

import concourse.bass as bass  # noqa: E402
import concourse.mybir as mybir  # noqa: E402
import concourse.tile as tile  # noqa: E402
from concourse import bacc  # noqa: E402
from concourse.bass_utils import run_bass_kernel_spmd  # noqa: E402
from concourse.masks import make_identity  # noqa: E402
from concourse.tile_rust import add_dep_helper  # noqa: E402

F32 = mybir.dt.float32
F32R = mybir.dt.float32r
BF16 = mybir.dt.bfloat16
F16 = mybir.dt.float16
F8 = mybir.dt.float8e4
ALU = mybir.AluOpType
AF = mybir.ActivationFunctionType

BETA = 0.9
THR = 1.0

# fp16 Dekker-split pre-scales (powers of 2; exact in fp32) and the evac
# scales that undo them during the PSUM->SBUF bias-add.
S_X = 16.0
S_W1 = 256.0
S_W2 = 256.0
SC1 = 1.0 / (S_X * S_W1)
SC2 = 1.0 / S_W2

B_FULL, T_FULL, D_FULL, H1_FULL, H2_FULL = 256, 64, 1024, 2048, 2048
N_CORES = 8
BL = B_FULL // N_CORES  # 32


def build_snn(T=T_FULL, D=D_FULL, H1=H1_FULL, H2=H2_FULL, T_NB=16):
    """Build the single-core Bass program (identical across the 8 cores)."""
    P = 128
    KC1 = D // P
    HC1 = H1 // P
    HC2 = H2 // P
    NNB = T // T_NB
    SUB = min(4, T_NB)
    NSUB = T_NB // SUB
    MCQ = min(4, HC2)
    HCQ = min(4, HC1)
    NB32 = T_NB * 32          # matmul free dim per t-block

    assert T % T_NB == 0 and T_NB % SUB == 0
    assert HC2 % MCQ == 0 and HC1 % HCQ == 0

    nc = bacc.Bacc("TRN2", target_bir_lowering=False, debug=False)

    xt_d = nc.dram_tensor("xThl", [D, 2, T * BL], F16, kind="ExternalInput")
    w1t_d = nc.dram_tensor("W1Thl", [D, 2, H1], F16, kind="ExternalInput")
    b1_d = nc.dram_tensor("b1", [H1], F32, kind="ExternalInput")
    w2t_d = nc.dram_tensor("W2Thl", [H1, 2, H2], F16, kind="ExternalInput")
    b2_d = nc.dram_tensor("b2", [H2], F32, kind="ExternalInput")

    spk2_d = nc.dram_tensor("spk2", [BL, H2], F32, kind="ExternalOutput")
    mem1_d = nc.dram_tensor("mem1", [BL, H1], F32, kind="ExternalOutput")
    mem2_d = nc.dram_tensor("mem2", [BL, H2], F32, kind="ExternalOutput")

    with tile.TileContext(nc) as tc:
        from contextlib import ExitStack
        ctx = ExitStack()
        with ctx:
            const = ctx.enter_context(tc.tile_pool(name="const", bufs=1))
            xtp = ctx.enter_context(tc.tile_pool(name="xtp", bufs=2))
            w1tp = ctx.enter_context(tc.tile_pool(name="w1tp", bufs=5))
            w2tp = ctx.enter_context(tc.tile_pool(name="w2tp", bufs=4))
            curp = ctx.enter_context(tc.tile_pool(name="curp", bufs=6))
            spk1p = ctx.enter_context(tc.tile_pool(name="spk1p", bufs=1))
            statep = ctx.enter_context(tc.tile_pool(name="statep", bufs=2))
            negzp = ctx.enter_context(tc.tile_pool(name="negzp", bufs=1))
            outp = ctx.enter_context(tc.tile_pool(name="outp", bufs=4))
            tpsum = ctx.enter_context(
                tc.tile_pool(name="tpsum", bufs=2, space="PSUM"))
            mpsum = ctx.enter_context(
                tc.tile_pool(name="mpsum", bufs=6, space="PSUM"))

            ident = const.tile([P, P], F32, name="ident")
            make_identity(nc, ident)

            # PE phase chaining (order-only deps): keeps fp32-mm, bf16-mm
            # and transpose phases from interleaving in the PE stream.
            pe_phases = []

            class _Ph:
                def __init__(self):
                    self.insts = []

                def add(self, bi):
                    self.insts.append(bi.ins)

            b1s = const.tile([P, HC1], F32, name="b1s")
            nc.gpsimd.dma_start(
                b1s[:], b1_d.ap().rearrange("(c p) -> p c", p=P))
            b2s = const.tile([P, HC2], F32, name="b2s")
            nc.gpsimd.dma_start(
                b2s[:], b2_d.ap().rearrange("(c p) -> p c", p=P))

            # ---------------- PE warmup (HAM ramp) --------------------------
            wub = const.tile([P, 256], BF16, name="wub")
            nc.vector.memset(wub[:], 0.0)
            wuw = const.tile([P, P], BF16, name="wuw")
            nc.vector.memset(wuw[:], 0.0)
            ph = _Ph()
            pe_phases.append(ph)
            wups = mpsum.tile([P, NB32], F32, tag="mm", name="wups")
            for i in range(20):
                ph.add(nc.tensor.matmul(wups[:, 0:256], wuw[:], wub[:],
                                        start=(i == 0), stop=(i == 19)))

            # ---------------- initial LIF state ----------------------------
            mem1_cur = statep.tile([P, HC1, 32], F32, tag="mem1",
                                   name="mem1_0")
            nc.vector.memset(mem1_cur[:], 0.0)
            mem2_cur = statep.tile([P, HC2, 32], F32, tag="mem2",
                                   name="mem2_0")
            nc.vector.memset(mem2_cur[:], 0.0)
            spk2_fin = const.tile([P, HC2, 32], F32, name="spk2_fin")

            # ---------------- outputs helper --------------------------------
            def emit_out(state, nch, out_d):
                ph = _Ph()
                pe_phases.append(ph)
                for hc in range(nch):
                    ps = tpsum.tile([32, P], F32, tag="tp", name="ops")
                    ph.add(nc.tensor.transpose(ps[:], state[:, hc, :],
                                               ident[:]))
                    sb = outp.tile([32, P], F32, tag="osb", name="osb")
                    nc.scalar.activation(sb[:], ps[:], AF.Copy)
                    nc.sync.dma_start(
                        out_d.ap()[:, hc * P:(hc + 1) * P], sb[:])

            # ---------------- per-block emitters ----------------------------
            def x_and_mm1(nb):
                """xT load + matmul1 for block nb -> cur1_subs"""
                ph = _Ph()
                pe_phases.append(ph)
                t0 = nb * T_NB
                xt = xtp.tile([P, KC1, 2, NB32], F16, tag="xt", name="xt")
                # block 0 is latency-critical at startup: spread its 8 chunk
                # loads over two otherwise-idle queues (weights own sync/
                # scalar); later blocks prefetch leisurely on gpsimd.
                for kc in range(KC1):
                    xq = (nc.gpsimd if kc % 2 == 0 else nc.scalar) \
                        if nb == 0 else nc.gpsimd
                    xq.dma_start(
                        xt[:, kc, :, :],
                        xt_d.ap()[kc * P:(kc + 1) * P, :,
                                  t0 * 32:(t0 + T_NB) * 32])

                cur1_subs = [curp.tile([P, SUB, HC1, 32], F32, tag="cur1",
                                       bufs=4, name="cur1")
                             for _ in range(NSUB)]
                for hq in range(HC1 // HCQ):
                    pss = [mpsum.tile([P, NB32], F32, tag="mm", name="mm1ps")
                           for _ in range(HCQ)]
                    for kc in range(KC1):
                        w1tt = w1tp.tile([P, 2, HCQ * P], F16, tag="w1t",
                                         name="w1tt")
                        dq = nc.sync if kc % 2 == 0 else nc.scalar
                        dq.dma_start(
                            w1tt[:],
                            w1t_d.ap()[kc * P:(kc + 1) * P, :,
                                       hq * HCQ * P:(hq + 1) * HCQ * P])
                        rhs_h = xt[:, kc, 0, :]
                        rhs_l = xt[:, kc, 1, :]
                        for i in range(HCQ):
                            # W1h*xh + W1h*xl + W1l*xh (~22-bit effective);
                            # consecutive same-stationary passes share LDW.
                            ph.add(nc.tensor.matmul(
                                pss[i][:], w1tt[:, 0, i * P:(i + 1) * P],
                                rhs_h, start=(kc == 0), stop=False))
                            ph.add(nc.tensor.matmul(
                                pss[i][:], w1tt[:, 0, i * P:(i + 1) * P],
                                rhs_l, start=False, stop=False))
                            ph.add(nc.tensor.matmul(
                                pss[i][:], w1tt[:, 1, i * P:(i + 1) * P],
                                rhs_h, start=False, stop=(kc == KC1 - 1)))
                    for s in range(NSUB):
                        for i in range(HCQ):
                            hc = hq * HCQ + i
                            psv = pss[i].rearrange("p (t b) -> p t b", b=32)
                            nc.scalar.activation(
                                cur1_subs[s][:, :, hc, :],
                                psv[:, s * SUB:(s + 1) * SUB, :],
                                AF.Identity, bias=b1s[:, hc:hc + 1],
                                scale=SC1)
                return cur1_subs

            # ---------------- scan emitters ---------------------------------
            def scan1(cur1_subs):
                """LIF-1 scan (T_NB steps) -> spk1 tile [(kc,t,b)] (fp16)."""
                nonlocal mem1_cur
                spk1 = spk1p.tile([P, HC1, NB32], F16, tag="spk1", bufs=2,
                                  name="spk1")
                for tr in range(T_NB):
                    cur_t = cur1_subs[tr // SUB][:, tr % SUB]  # [P, HC1, 32]
                    negz = negzp.tile([P, HC1, 32], F32, tag="negz",
                                      name="negz")
                    nc.vector.scalar_tensor_tensor(
                        negz[:], mem1_cur[:], THR, cur_t,
                        ALU.is_gt, ALU.subtract)
                    mem1_new = statep.tile([P, HC1, 32], F32, tag="mem1",
                                           name="mem1")
                    nc.vector.scalar_tensor_tensor(
                        mem1_new[:], mem1_cur[:], BETA, negz[:],
                        ALU.mult, ALU.subtract)
                    mem1_cur = mem1_new
                    # spike of step t thresholds the POST-update membrane
                    nc.vector.tensor_scalar(
                        spk1[:, :, tr * 32:(tr + 1) * 32], mem1_cur[:],
                        THR, None, ALU.is_gt)
                return spk1

            def mm2(spk1, halves=1):
                """cur2[(t,mc,b)] = W2 @ spk1^T + b2 (2x fp16 passes).
                halves=2 runs the free dim in two column halves (whole mq
                sweep per half) so the following scan2 can start at the
                midpoint -- used for the last block, whose scan2 tail has no
                mm1 behind it to hide under."""
                ph = _Ph()
                pe_phases.append(ph)
                cur2_subs = [curp.tile([P, SUB, HC2, 32], F32, tag="cur2",
                                       bufs=4, name="cur2")
                             for _ in range(NSUB)]
                HW = NB32 // halves
                SH = NSUB // halves
                for h in range(halves):
                    for mq in range(HC2 // MCQ):
                        pss = [mpsum.tile([P, NB32], F32, tag="mm",
                                          name="mm2ps")
                               for _ in range(MCQ)]
                        for kc in range(HC1):
                            wt = w2tp.tile([P, 2, MCQ * P], F16, tag="w2t",
                                           name="w2t")
                            dq = nc.sync if kc % 2 == 0 else nc.scalar
                            dq.dma_start(
                                wt[:],
                                w2t_d.ap()[kc * P:(kc + 1) * P, :,
                                           mq * MCQ * P:(mq + 1) * MCQ * P])
                            rhs = spk1[:, kc, h * HW:(h + 1) * HW]
                            for i in range(MCQ):
                                ph.add(nc.tensor.matmul(
                                    pss[i][:, 0:HW],
                                    wt[:, 0, i * P:(i + 1) * P], rhs,
                                    start=(kc == 0), stop=False))
                                ph.add(nc.tensor.matmul(
                                    pss[i][:, 0:HW],
                                    wt[:, 1, i * P:(i + 1) * P], rhs,
                                    start=False, stop=(kc == HC1 - 1)))
                        # sub-major evac order so scan2's next inputs (s of
                        # every mc) complete as early as possible
                        for sl in range(SH):
                            s = h * SH + sl
                            for i in range(MCQ):
                                mc = mq * MCQ + i
                                psv = pss[i].rearrange("p (t b) -> p t b",
                                                       b=32)
                                nc.scalar.activation(
                                    cur2_subs[s][:, :, mc, :],
                                    psv[:, sl * SUB:(sl + 1) * SUB, :],
                                    AF.Identity, bias=b2s[:, mc:mc + 1],
                                    scale=SC2)
                return cur2_subs

            def scan2(cur2_subs, nb):
                nonlocal mem2_cur
                t0 = nb * T_NB
                for tr in range(T_NB):
                    t = t0 + tr
                    cur_t = cur2_subs[tr // SUB][:, tr % SUB]
                    negz = negzp.tile([P, HC2, 32], F32, tag="negz",
                                      name="negz")
                    nc.vector.scalar_tensor_tensor(
                        negz[:], mem2_cur[:], THR, cur_t,
                        ALU.is_gt, ALU.subtract)
                    mem2_new = statep.tile([P, HC2, 32], F32, tag="mem2",
                                           name="mem2")
                    nc.vector.scalar_tensor_tensor(
                        mem2_new[:], mem2_cur[:], BETA, negz[:],
                        ALU.mult, ALU.subtract)
                    mem2_cur = mem2_new
                    if t == T - 1:
                        nc.vector.tensor_scalar(
                            spk2_fin[:], mem2_cur[:], THR, None, ALU.is_gt)

            # ---------------- main t-block pipeline -------------------------
            # Software pipelining, two levels:
            #  - PE stream: mm1(nb+1) is emitted BEFORE mm2(nb) so the PE
            #    fills the scan1 latency instead of stalling on spk1.
            #  - DVE stream: scan1(nb+1) is emitted BEFORE scan2(nb) so the
            #    (FIFO) vector engine runs scan1(nb+1) during mm2(nb) instead
            #    of queuing it behind scan2(nb), which can only start once
            #    mm2(nb) is nearly done. This keeps spk1(nb+1) ready the
            #    moment mm2(nb) retires -- critical for the last block, where
            #    no mm1(nb+1) exists to hide the wait.
            cur1_next = x_and_mm1(0)
            spk1_next = scan1(cur1_next)
            for nb in range(NNB):
                spk1_cur = spk1_next
                if nb + 1 < NNB:
                    cur1_next = x_and_mm1(nb + 1)
                if nb == NNB - 1:
                    emit_out(mem1_cur, HC1, mem1_d)
                cur2_subs = mm2(spk1_cur, halves=(2 if nb == NNB - 1 else 1))
                if nb + 1 < NNB:
                    spk1_next = scan1(cur1_next)
                scan2(cur2_subs, nb)

            # ---------------- remaining outputs -----------------------------
            emit_out(mem2_cur, HC2, mem2_d)
            emit_out(spk2_fin, HC2, spk2_d)

            # chain consecutive PE phases: every inst of phase b ordered
            # after the last inst of phase a (order-only deps)
            for a, b in zip(pe_phases, pe_phases[1:]):
                if a.insts and b.insts:
                    for bi in b.insts:
                        add_dep_helper(bi, a.insts[-1], sync=False,
                                       reason="PE phase ordering")

    nc.compile()
    return nc


_NC_CACHE = {}


def _get_nc():
    if "full" not in _NC_CACHE:
        _NC_CACHE["full"] = build_snn()
    return _NC_CACHE["full"]


def _dekker_f16(a):
    """Split fp32 array into fp16 hi+lo terms stacked on axis 1."""
    hi = a.astype(np.float16)
    lo = (a - hi.astype(np.float32)).astype(np.float16)
    return np.ascontiguousarray(np.stack([hi, lo], axis=1))


def prep_inputs(x, W1, b1, W2, b2):
    """Host-side prep: shard x over cores (transposed to [d, (t,b)]) and
    Dekker-split x, W1, W2 into pre-scaled fp16 hi+lo pairs."""
    x = np.asarray(x, np.float32)
    W1 = np.asarray(W1, np.float32)
    b1 = np.ascontiguousarray(np.asarray(b1, np.float32))
    W2 = np.asarray(W2, np.float32)
    b2 = np.ascontiguousarray(np.asarray(b2, np.float32))
    B, T, D = x.shape

    W1Thl = _dekker_f16(W1.T * np.float32(S_W1))        # [D, 2, H1]
    W2Thl = _dekker_f16(W2.T * np.float32(S_W2))        # [H1, 2, H2]

    bl = B // N_CORES
    in_maps = []
    for c in range(N_CORES):
        xc = x[c * bl:(c + 1) * bl]                     # [bl, T, D]
        xT = xc.transpose(2, 1, 0).reshape(D, T * bl)   # [d, (t,b)] t-major
        xThl = _dekker_f16(xT * np.float32(S_X))        # [D, 2, (t,b)]
        in_maps.append({
            "xThl": xThl, "W1Thl": W1Thl, "b1": b1, "W2Thl": W2Thl,
            "b2": b2,
        })
    return in_maps


def kernel(x, W1, b1, W2, b2):
    """Full-input entry point: shards B across 8 NeuronCores, returns full
    (spk2, mem1, mem2) exactly like reference()."""
    nc = _get_nc()
    in_maps = prep_inputs(x, W1, b1, W2, b2)
    res = run_bass_kernel_spmd(nc, in_maps, core_ids=list(range(N_CORES)))
    spk2 = np.concatenate([res.results[c]["spk2"] for c in range(N_CORES)], 0)
    mem1 = np.concatenate([res.results[c]["mem1"] for c in range(N_CORES)], 0)
    mem2 = np.concatenate([res.results[c]["mem2"] for c in range(N_CORES)], 0)
    return spk2, mem1, mem2



# revision 40
# speedup vs baseline: 1.1908x; 1.1908x over previous
"""Feedforward SNN (Linear -> LIF) x2 kernel for Trainium2, 8-core data parallel.

Per-core plan (B sharded 8 ways, BL=32 samples/core):
  - Host pre-transposes operands once (cheap numpy) and Dekker-splits EVERY
    matmul operand into fp16 hi+lo pairs (pre-scaled by powers of 2 to dodge
    fp16 subnormals). fp16 runs the PE at 1.0 cycles/row (vs fp32's 4.0) and
    its 11-bit mantissa is held exactly by the PE's internal FP22 format, so
    a 2-term split carries ~22 significant bits -- fp32-grade for this model
    (validated: end-to-end error below the fp32 BLAS-reorder noise floor).
  - Layer-1 currents for ALL timesteps: Cur1[h1, (t,b)] = W1 @ x^T as THREE
    fp16 matmuls (W1h*xh + W1l*xh + W1h*xl; the dropped W1l*xl term is
    ~2^-22 relative). 3 passes at 1x rate vs fp32's 4x-slow single pass.
  - LIF-1 scan over t on [128, HC1*32] tiles (partition = h1 % 128, free =
    (h1chunk, b)); fused scalar_tensor_tensor DVE ops, 3/step.
  - Spikes are {0,1} == exact in fp16; layer-2 currents are 2x fp16 matmuls
    (W2h + W2l = 22-bit W2) accumulated in fp32 PSUM.
  - The pre-scales are undone for free in the PSUM->SBUF bias-add
    activation (out = psum*scale + bias).
  - LIF-2 scan likewise (2 DVE ops/step; spikes materialized only at t=63).
  - Software-pipelined: mm1(nb+1) is emitted before mm2(nb) so the PE fills
    the scan1(nb) latency; PE phases are chained with order-only deps.
"""

import os
import sys

import numpy as np

for _p in ("/opt/trn_rl_repo", "/root/.axon_site/_ro/trn_rl_repo"):
    if os.path.isdir(_p) and _p not in sys.path:
        sys.path.insert(0, _p)

import ml_dtypes  # noqa: E402

import concourse.bass as bass  # noqa: E402
import concourse.mybir as mybir  # noqa: E402
import concourse.tile as tile  # noqa: E402
from concourse import bacc  # noqa: E402
from concourse.bass_utils import run_bass_kernel_spmd  # noqa: E402
from concourse.masks import make_identity  # noqa: E402
from concourse.tile_rust import add_dep_helper  # noqa: E402

F32 = mybir.dt.float32
F32R = mybir.dt.float32r
BF16 = mybir.dt.bfloat16
F16 = mybir.dt.float16
F8 = mybir.dt.float8e4
ALU = mybir.AluOpType
AF = mybir.ActivationFunctionType

BETA = 0.9
THR = 1.0

# fp16 Dekker-split pre-scales (powers of 2; exact in fp32) and the evac
# scales that undo them during the PSUM->SBUF bias-add.
S_X = 16.0
S_W1 = 256.0
S_W2 = 256.0
SC1 = 1.0 / (S_X * S_W1)
SC2 = 1.0 / S_W2

B_FULL, T_FULL, D_FULL, H1_FULL, H2_FULL = 256, 64, 1024, 2048, 2048
N_CORES = 8
BL = B_FULL // N_CORES  # 32


def build_snn(T=T_FULL, D=D_FULL, H1=H1_FULL, H2=H2_FULL, T_NB=16):
    """Build the single-core Bass program (identical across the 8 cores)."""
    P = 128
    KC1 = D // P
    HC1 = H1 // P
    HC2 = H2 // P
    NNB = T // T_NB
    SUB = min(4, T_NB)
    NSUB = T_NB // SUB
    MCQ = min(4, HC2)
    HCQ = min(4, HC1)
    NB32 = T_NB * 32          # matmul free dim per t-block

    assert T % T_NB == 0 and T_NB % SUB == 0
    assert HC2 % MCQ == 0 and HC1 % HCQ == 0

    nc = bacc.Bacc("TRN2", target_bir_lowering=False, debug=False)

    xt_d = nc.dram_tensor("xThl", [D, 2, T * BL], F16, kind="ExternalInput")
    w1t_d = nc.dram_tensor("W1Thl", [D, 2, H1], F16, kind="ExternalInput")
    b1_d = nc.dram_tensor("b1", [H1], F32, kind="ExternalInput")
    # W2 = fp16 hi (11 bits) + one fp8 residual term consumed by a DoubleRow
    # matmul (2x K per instruction).  W2l8 is in DR pair layout: row
    # (q*128+p) holds the pair h1=(2q+j)*128+p along j, scaled 2^8 so the
    # residual sits in e4m3's normal range; the matching spike tensor is
    # {0, 2^-8} so the products land at the same scale as the hi pass and
    # share its PSUM accumulation.
    w2t_d = nc.dram_tensor("W2Th", [H1, H2], F16, kind="ExternalInput")
    w2l_d = nc.dram_tensor("W2l8", [H1 // 2, 2, H2], F8,
                           kind="ExternalInput")
    b2_d = nc.dram_tensor("b2", [H2], F32, kind="ExternalInput")

    spk2_d = nc.dram_tensor("spk2", [BL, H2], F32, kind="ExternalOutput")
    mem1_d = nc.dram_tensor("mem1", [BL, H1], F32, kind="ExternalOutput")
    mem2_d = nc.dram_tensor("mem2", [BL, H2], F32, kind="ExternalOutput")

    with tile.TileContext(nc) as tc:
        from contextlib import ExitStack
        ctx = ExitStack()
        with ctx:
            const = ctx.enter_context(tc.tile_pool(name="const", bufs=1))
            xtp = ctx.enter_context(tc.tile_pool(name="xtp", bufs=2))
            w1tp = ctx.enter_context(tc.tile_pool(name="w1tp", bufs=8))
            w2tp = ctx.enter_context(tc.tile_pool(name="w2tp", bufs=8))
            w2lp = ctx.enter_context(tc.tile_pool(name="w2lp", bufs=8))
            curp = ctx.enter_context(tc.tile_pool(name="curp", bufs=6))
            spk1p = ctx.enter_context(tc.tile_pool(name="spk1p", bufs=1))
            statep = ctx.enter_context(tc.tile_pool(name="statep", bufs=2))
            negzp = ctx.enter_context(tc.tile_pool(name="negzp", bufs=1))
            outp = ctx.enter_context(tc.tile_pool(name="outp", bufs=4))
            tpsum = ctx.enter_context(
                tc.tile_pool(name="tpsum", bufs=2, space="PSUM"))
            mpsum = ctx.enter_context(
                tc.tile_pool(name="mpsum", bufs=6, space="PSUM"))

            ident = const.tile([P, P], F32, name="ident")
            make_identity(nc, ident)

            # PE phase chaining (order-only deps): keeps fp32-mm, bf16-mm
            # and transpose phases from interleaving in the PE stream.
            pe_phases = []

            class _Ph:
                def __init__(self):
                    self.insts = []

                def add(self, bi):
                    self.insts.append(bi.ins)

            b1s = const.tile([P, HC1], F32, name="b1s")
            nc.gpsimd.dma_start(
                b1s[:], b1_d.ap().rearrange("(c p) -> p c", p=P))
            b2s = const.tile([P, HC2], F32, name="b2s")
            nc.gpsimd.dma_start(
                b2s[:], b2_d.ap().rearrange("(c p) -> p c", p=P))

            # ---------------- PE warmup (HAM ramp) --------------------------
            wub = const.tile([P, 256], BF16, name="wub")
            nc.vector.memset(wub[:], 0.0)
            wuw = const.tile([P, P], BF16, name="wuw")
            nc.vector.memset(wuw[:], 0.0)
            ph = _Ph()
            pe_phases.append(ph)
            wups = mpsum.tile([P, NB32], F32, tag="mm", name="wups")
            for i in range(20):
                ph.add(nc.tensor.matmul(wups[:, 0:256], wuw[:], wub[:],
                                        start=(i == 0), stop=(i == 19)))

            # ---------------- initial LIF state ----------------------------
            mem1_cur = statep.tile([P, HC1, 32], F32, tag="mem1",
                                   name="mem1_0")
            nc.vector.memset(mem1_cur[:], 0.0)
            mem2_cur = statep.tile([P, HC2, 32], F32, tag="mem2",
                                   name="mem2_0")
            nc.vector.memset(mem2_cur[:], 0.0)
            spk2_fin = const.tile([P, HC2, 32], F32, name="spk2_fin")

            # ---------------- outputs helper --------------------------------
            def emit_out(state, nch, out_d):
                ph = _Ph()
                pe_phases.append(ph)
                for hc in range(nch):
                    ps = tpsum.tile([32, P], F32, tag="tp", name="ops")
                    ph.add(nc.tensor.transpose(ps[:], state[:, hc, :],
                                               ident[:]))
                    sb = outp.tile([32, P], F32, tag="osb", name="osb")
                    nc.scalar.activation(sb[:], ps[:], AF.Copy)
                    nc.sync.dma_start(
                        out_d.ap()[:, hc * P:(hc + 1) * P], sb[:])

            # ---------------- per-block emitters ----------------------------
            def x_and_mm1(nb):
                """xT load + matmul1 for block nb -> cur1_subs"""
                ph = _Ph()
                pe_phases.append(ph)
                t0 = nb * T_NB
                xt = xtp.tile([P, KC1, 2, NB32], F16, tag="xt", name="xt")
                # block 0 is latency-critical at startup: spread its 8 chunk
                # loads over two otherwise-idle queues (weights own sync/
                # scalar); later blocks prefetch leisurely on gpsimd.
                for kc in range(KC1):
                    xq = (nc.gpsimd if kc % 2 == 0 else nc.scalar) \
                        if nb == 0 else nc.gpsimd
                    xq.dma_start(
                        xt[:, kc, :, :],
                        xt_d.ap()[kc * P:(kc + 1) * P, :,
                                  t0 * 32:(t0 + T_NB) * 32])

                cur1_subs = [curp.tile([P, SUB, HC1, 32], F32, tag="cur1",
                                       bufs=6, name="cur1")
                             for _ in range(NSUB)]
                for hq in range(HC1 // HCQ):
                    pss = [mpsum.tile([P, NB32], F32, tag="mm", name="mm1ps")
                           for _ in range(HCQ)]
                    for kc in range(KC1):
                        w1tt = w1tp.tile([P, 2, HCQ * P], F16, tag="w1t",
                                         name="w1tt")
                        dq = nc.sync if kc % 2 == 0 else nc.scalar
                        dq.dma_start(
                            w1tt[:],
                            w1t_d.ap()[kc * P:(kc + 1) * P, :,
                                       hq * HCQ * P:(hq + 1) * HCQ * P])
                        rhs_h = xt[:, kc, 0, :]
                        rhs_l = xt[:, kc, 1, :]
                        for i in range(HCQ):
                            # W1h*xh + W1h*xl + W1l*xh (~22-bit effective);
                            # consecutive same-stationary passes share LDW.
                            ph.add(nc.tensor.matmul(
                                pss[i][:], w1tt[:, 0, i * P:(i + 1) * P],
                                rhs_h, start=(kc == 0), stop=False))
                            ph.add(nc.tensor.matmul(
                                pss[i][:], w1tt[:, 0, i * P:(i + 1) * P],
                                rhs_l, start=False, stop=False))
                            ph.add(nc.tensor.matmul(
                                pss[i][:], w1tt[:, 1, i * P:(i + 1) * P],
                                rhs_h, start=False, stop=(kc == KC1 - 1)))
                    for s in range(NSUB):
                        for i in range(HCQ):
                            hc = hq * HCQ + i
                            psv = pss[i].rearrange("p (t b) -> p t b", b=32)
                            nc.scalar.activation(
                                cur1_subs[s][:, :, hc, :],
                                psv[:, s * SUB:(s + 1) * SUB, :],
                                AF.Identity, bias=b1s[:, hc:hc + 1],
                                scale=SC1)
                return cur1_subs

            # ---------------- scan emitters ---------------------------------
            def scan1(cur1_subs):
                """LIF-1 scan (T_NB steps) -> (fp16 {0,1} spikes,
                fp8 {0,2^-8} spikes) tiles, both [(kc,t,b)]."""
                nonlocal mem1_cur
                spk1 = spk1p.tile([P, HC1, NB32], F16, tag="spk1", bufs=2,
                                  name="spk1")
                spk8 = spk1p.tile([P, HC1, NB32], F8, tag="spk8", bufs=2,
                                  name="spk8")
                for tr in range(T_NB):
                    cur_t = cur1_subs[tr // SUB][:, tr % SUB]  # [P, HC1, 32]
                    negz = negzp.tile([P, HC1, 32], F32, tag="negz",
                                      name="negz")
                    nc.vector.scalar_tensor_tensor(
                        negz[:], mem1_cur[:], THR, cur_t,
                        ALU.is_gt, ALU.subtract)
                    mem1_new = statep.tile([P, HC1, 32], F32, tag="mem1",
                                           name="mem1")
                    nc.vector.scalar_tensor_tensor(
                        mem1_new[:], mem1_cur[:], BETA, negz[:],
                        ALU.mult, ALU.subtract)
                    mem1_cur = mem1_new
                    # spike of step t thresholds the POST-update membrane
                    nc.vector.tensor_scalar(
                        spk1[:, :, tr * 32:(tr + 1) * 32], mem1_cur[:],
                        THR, None, ALU.is_gt)
                    nc.vector.tensor_scalar(
                        spk8[:, :, tr * 32:(tr + 1) * 32], mem1_cur[:],
                        THR, 1.0 / 256.0, ALU.is_gt, ALU.mult)
                return spk1, spk8

            def mm2(spks):
                """cur2[(t,mc,b)] = W2 @ spk1^T + b2: one fp16 hi pass +
                one fp8 DoubleRow residual pass (K=256/inst), same PSUM."""
                spk1, spk8 = spks
                ph = _Ph()
                pe_phases.append(ph)
                cur2_subs = [curp.tile([P, SUB, HC2, 32], F32, tag="cur2",
                                       bufs=4, name="cur2")
                             for _ in range(NSUB)]
                KP = HC1 // 2
                for mq in range(HC2 // MCQ):
                    pss = [mpsum.tile([P, NB32], F32, tag="mm",
                                      name="mm2ps")
                           for _ in range(MCQ)]
                    for kc in range(HC1):
                        wt = w2tp.tile([P, MCQ * P], F16, tag="w2t",
                                       name="w2t")
                        dq = nc.sync if kc % 2 == 0 else nc.scalar
                        dq.dma_start(
                            wt[:],
                            w2t_d.ap()[kc * P:(kc + 1) * P,
                                       mq * MCQ * P:(mq + 1) * MCQ * P])
                        rhs = spk1[:, kc, :]
                        for i in range(MCQ):
                            ph.add(nc.tensor.matmul(
                                pss[i][:], wt[:, i * P:(i + 1) * P], rhs,
                                start=(kc == 0), stop=False))
                    for q in range(KP):
                        wl = w2lp.tile([P, 2, MCQ * P], F8, tag="w2l",
                                       name="w2l")
                        nc.gpsimd.dma_start(
                            wl[:],
                            w2l_d.ap()[q * P:(q + 1) * P, :,
                                       mq * MCQ * P:(mq + 1) * MCQ * P])
                        rhs = spk8[:, 2 * q:2 * q + 2, :]
                        for i in range(MCQ):
                            ph.add(nc.tensor.matmul(
                                pss[i][:], wl[:, :, i * P:(i + 1) * P], rhs,
                                start=False, stop=(q == KP - 1),
                                perf_mode=mybir.MatmulPerfMode.DoubleRow))
                    # sub-major evac order so scan2 step 0's inputs (s=0 of
                    # every mc) complete as early as possible
                    for s in range(NSUB):
                        for i in range(MCQ):
                            mc = mq * MCQ + i
                            psv = pss[i].rearrange("p (t b) -> p t b", b=32)
                            nc.scalar.activation(
                                cur2_subs[s][:, :, mc, :],
                                psv[:, s * SUB:(s + 1) * SUB, :],
                                AF.Identity, bias=b2s[:, mc:mc + 1],
                                scale=SC2)
                return cur2_subs

            def scan2(cur2_subs, nb):
                nonlocal mem2_cur
                t0 = nb * T_NB
                for tr in range(T_NB):
                    t = t0 + tr
                    cur_t = cur2_subs[tr // SUB][:, tr % SUB]
                    negz = negzp.tile([P, HC2, 32], F32, tag="negz",
                                      name="negz")
                    nc.vector.scalar_tensor_tensor(
                        negz[:], mem2_cur[:], THR, cur_t,
                        ALU.is_gt, ALU.subtract)
                    mem2_new = statep.tile([P, HC2, 32], F32, tag="mem2",
                                           name="mem2")
                    nc.vector.scalar_tensor_tensor(
                        mem2_new[:], mem2_cur[:], BETA, negz[:],
                        ALU.mult, ALU.subtract)
                    mem2_cur = mem2_new
                    if t == T - 1:
                        nc.vector.tensor_scalar(
                            spk2_fin[:], mem2_cur[:], THR, None, ALU.is_gt)

            # ---------------- main t-block pipeline -------------------------
            # Software pipelining, two levels:
            #  - PE stream: mm1(nb+1) is emitted BEFORE mm2(nb) so the PE
            #    fills the scan1 latency instead of stalling on spk1.
            #  - DVE stream: scan1(nb+1) is emitted BEFORE scan2(nb) so the
            #    (FIFO) vector engine runs scan1(nb+1) during mm2(nb) instead
            #    of queuing it behind scan2(nb), which can only start once
            #    mm2(nb) is nearly done. This keeps spk1(nb+1) ready the
            #    moment mm2(nb) retires -- critical for the last block, where
            #    no mm1(nb+1) exists to hide the wait.
            cur1_next = x_and_mm1(0)
            spk1_next = scan1(cur1_next)
            for nb in range(NNB):
                spk1_cur = spk1_next
                if nb + 1 < NNB:
                    cur1_next = x_and_mm1(nb + 1)
                if nb == NNB - 1:
                    emit_out(mem1_cur, HC1, mem1_d)
                cur2_subs = mm2(spk1_cur)
                if nb + 1 < NNB:
                    spk1_next = scan1(cur1_next)
                scan2(cur2_subs, nb)

            # ---------------- remaining outputs -----------------------------
            emit_out(mem2_cur, HC2, mem2_d)
            emit_out(spk2_fin, HC2, spk2_d)

            # chain consecutive PE phases: every inst of phase b ordered
            # after the last inst of phase a (order-only deps)
            for a, b in zip(pe_phases, pe_phases[1:]):
                if a.insts and b.insts:
                    for bi in b.insts:
                        add_dep_helper(bi, a.insts[-1], sync=False,
                                       reason="PE phase ordering")

    nc.compile()
    return nc


_NC_CACHE = {}


def _get_nc():
    if "full" not in _NC_CACHE:
        _NC_CACHE["full"] = build_snn()
    return _NC_CACHE["full"]


def _dekker_f16(a):
    """Split fp32 array into fp16 hi+lo terms stacked on axis 1."""
    hi = a.astype(np.float16)
    lo = (a - hi.astype(np.float32)).astype(np.float16)
    return np.ascontiguousarray(np.stack([hi, lo], axis=1))


def prep_inputs(x, W1, b1, W2, b2):
    """Host-side prep: shard x over cores (transposed to [d, (t,b)]) and
    Dekker-split x, W1, W2 into pre-scaled fp16 hi+lo pairs."""
    x = np.asarray(x, np.float32)
    W1 = np.asarray(W1, np.float32)
    b1 = np.ascontiguousarray(np.asarray(b1, np.float32))
    W2 = np.asarray(W2, np.float32)
    b2 = np.ascontiguousarray(np.asarray(b2, np.float32))
    B, T, D = x.shape

    W1Thl = _dekker_f16(W1.T * np.float32(S_W1))        # [D, 2, H1]

    # W2 -> fp16 hi + fp8 residual (scaled 2^8) in DoubleRow pair layout
    H1 = W2.shape[0]
    W2s = np.ascontiguousarray(W2.T) * np.float32(S_W2)  # [H1, H2]
    W2Th = W2s.astype(np.float16)
    r = (W2s - W2Th.astype(np.float32)) * np.float32(256.0)
    W2l8 = np.ascontiguousarray(
        r.astype(ml_dtypes.float8_e4m3)
        .reshape(H1 // 256, 2, 128, -1)
        .transpose(0, 2, 1, 3)
        .reshape(H1 // 2, 2, -1))                       # [(q p), j, H2]

    bl = B // N_CORES
    in_maps = []
    for c in range(N_CORES):
        xc = x[c * bl:(c + 1) * bl]                     # [bl, T, D]
        xT = xc.transpose(2, 1, 0).reshape(D, T * bl)   # [d, (t,b)] t-major
        xThl = _dekker_f16(xT * np.float32(S_X))        # [D, 2, (t,b)]
        in_maps.append({
            "xThl": xThl, "W1Thl": W1Thl, "b1": b1, "W2Th": W2Th,
            "W2l8": W2l8, "b2": b2,
        })
    return in_maps


def kernel(x, W1, b1, W2, b2):
    """Full-input entry point: shards B across 8 NeuronCores, returns full
    (spk2, mem1, mem2) exactly like reference()."""
    nc = _get_nc()
    in_maps = prep_inputs(x, W1, b1, W2, b2)
    res = run_bass_kernel_spmd(nc, in_maps, core_ids=list(range(N_CORES)))
    spk2 = np.concatenate([res.results[c]["spk2"] for c in range(N_CORES)], 0)
    mem1 = np.concatenate([res.results[c]["mem1"] for c in range(N_CORES)], 0)
    mem2 = np.concatenate([res.results[c]["mem2"] for c in range(N_CORES)], 0)
    return spk2, mem1, mem2



# revision 41
# speedup vs baseline: 1.1986x; 1.0065x over previous
"""Feedforward SNN (Linear -> LIF) x2 kernel for Trainium2, 8-core data parallel.

Per-core plan (B sharded 8 ways, BL=32 samples/core):
  - Host pre-transposes operands once (cheap numpy) and Dekker-splits EVERY
    matmul operand into fp16 hi+lo pairs (pre-scaled by powers of 2 to dodge
    fp16 subnormals). fp16 runs the PE at 1.0 cycles/row (vs fp32's 4.0) and
    its 11-bit mantissa is held exactly by the PE's internal FP22 format, so
    a 2-term split carries ~22 significant bits -- fp32-grade for this model
    (validated: end-to-end error below the fp32 BLAS-reorder noise floor).
  - Layer-1 currents for ALL timesteps: Cur1[h1, (t,b)] = W1 @ x^T as THREE
    fp16 matmuls (W1h*xh + W1l*xh + W1h*xl; the dropped W1l*xl term is
    ~2^-22 relative). 3 passes at 1x rate vs fp32's 4x-slow single pass.
  - LIF-1 scan over t on [128, HC1*32] tiles (partition = h1 % 128, free =
    (h1chunk, b)); fused scalar_tensor_tensor DVE ops, 3/step.
  - Spikes are {0,1} == exact in fp16; layer-2 currents are 2x fp16 matmuls
    (W2h + W2l = 22-bit W2) accumulated in fp32 PSUM.
  - The pre-scales are undone for free in the PSUM->SBUF bias-add
    activation (out = psum*scale + bias).
  - LIF-2 scan likewise (2 DVE ops/step; spikes materialized only at t=63).
  - Software-pipelined: mm1(nb+1) is emitted before mm2(nb) so the PE fills
    the scan1(nb) latency; PE phases are chained with order-only deps.
"""

import os
import sys

import numpy as np

for _p in ("/opt/trn_rl_repo", "/root/.axon_site/_ro/trn_rl_repo"):
    if os.path.isdir(_p) and _p not in sys.path:
        sys.path.insert(0, _p)

import ml_dtypes  # noqa: E402

import concourse.bass as bass  # noqa: E402
import concourse.mybir as mybir  # noqa: E402
import concourse.tile as tile  # noqa: E402
from concourse import bacc  # noqa: E402
from concourse.bass_utils import run_bass_kernel_spmd  # noqa: E402
from concourse.masks import make_identity  # noqa: E402
from concourse.tile_rust import add_dep_helper  # noqa: E402

F32 = mybir.dt.float32
F32R = mybir.dt.float32r
BF16 = mybir.dt.bfloat16
F16 = mybir.dt.float16
F8 = mybir.dt.float8e4
ALU = mybir.AluOpType
AF = mybir.ActivationFunctionType

BETA = 0.9
THR = 1.0

# fp16 Dekker-split pre-scales (powers of 2; exact in fp32) and the evac
# scales that undo them during the PSUM->SBUF bias-add.
S_X = 16.0
S_W1 = 256.0
S_W2 = 256.0
SC1 = 1.0 / (S_X * S_W1)
SC2 = 1.0 / S_W2

B_FULL, T_FULL, D_FULL, H1_FULL, H2_FULL = 256, 64, 1024, 2048, 2048
N_CORES = 8
BL = B_FULL // N_CORES  # 32


def build_snn(T=T_FULL, D=D_FULL, H1=H1_FULL, H2=H2_FULL, T_NB=16):
    """Build the single-core Bass program (identical across the 8 cores)."""
    P = 128
    KC1 = D // P
    HC1 = H1 // P
    HC2 = H2 // P
    NNB = T // T_NB
    SUB = min(4, T_NB)
    NSUB = T_NB // SUB
    SUBG = min(8, T_NB)
    NSUBG = T_NB // SUBG
    MCQ = min(4, HC2)
    HCQ = min(4, HC1)
    NB32 = T_NB * 32          # matmul free dim per t-block

    assert T % T_NB == 0 and T_NB % SUB == 0
    assert HC2 % MCQ == 0 and HC1 % HCQ == 0

    nc = bacc.Bacc("TRN2", target_bir_lowering=False, debug=False)

    xt_d = nc.dram_tensor("xThl", [D, 2, T * BL], F16, kind="ExternalInput")
    w1t_d = nc.dram_tensor("W1Thl", [D, 2, H1], F16, kind="ExternalInput")
    b1_d = nc.dram_tensor("b1", [H1], F32, kind="ExternalInput")
    # W2 = fp16 hi (11 bits) + one fp8 residual term consumed by a DoubleRow
    # matmul (2x K per instruction).  W2l8 is in DR pair layout: row
    # (q*128+p) holds the pair h1=(2q+j)*128+p along j, scaled 2^8 so the
    # residual sits in e4m3's normal range; the matching spike tensor is
    # {0, 2^-8} so the products land at the same scale as the hi pass and
    # share its PSUM accumulation.
    w2t_d = nc.dram_tensor("W2Th", [H1, H2], F16, kind="ExternalInput")
    w2l_d = nc.dram_tensor("W2l8", [H1 // 2, 2, H2], F8,
                           kind="ExternalInput")
    b2_d = nc.dram_tensor("b2", [H2], F32, kind="ExternalInput")

    spk2_d = nc.dram_tensor("spk2", [BL, H2], F32, kind="ExternalOutput")
    mem1_d = nc.dram_tensor("mem1", [BL, H1], F32, kind="ExternalOutput")
    mem2_d = nc.dram_tensor("mem2", [BL, H2], F32, kind="ExternalOutput")

    with tile.TileContext(nc) as tc:
        from contextlib import ExitStack
        ctx = ExitStack()
        with ctx:
            const = ctx.enter_context(tc.tile_pool(name="const", bufs=1))
            xtp = ctx.enter_context(tc.tile_pool(name="xtp", bufs=2))
            w1tp = ctx.enter_context(tc.tile_pool(name="w1tp", bufs=8))
            w2tp = ctx.enter_context(tc.tile_pool(name="w2tp", bufs=8))
            w2lp = ctx.enter_context(tc.tile_pool(name="w2lp", bufs=8))
            curp = ctx.enter_context(tc.tile_pool(name="curp", bufs=6))
            spk1p = ctx.enter_context(tc.tile_pool(name="spk1p", bufs=1))
            statep = ctx.enter_context(tc.tile_pool(name="statep", bufs=2))
            negzp = ctx.enter_context(tc.tile_pool(name="negzp", bufs=1))
            outp = ctx.enter_context(tc.tile_pool(name="outp", bufs=4))
            tpsum = ctx.enter_context(
                tc.tile_pool(name="tpsum", bufs=2, space="PSUM"))
            mpsum = ctx.enter_context(
                tc.tile_pool(name="mpsum", bufs=6, space="PSUM"))

            ident = const.tile([P, P], F32, name="ident")
            make_identity(nc, ident)

            # PE phase chaining (order-only deps): keeps fp32-mm, bf16-mm
            # and transpose phases from interleaving in the PE stream.
            pe_phases = []

            class _Ph:
                def __init__(self):
                    self.insts = []

                def add(self, bi):
                    self.insts.append(bi.ins)

            b1s = const.tile([P, HC1], F32, name="b1s")
            nc.gpsimd.dma_start(
                b1s[:], b1_d.ap().rearrange("(c p) -> p c", p=P))
            b2s = const.tile([P, HC2], F32, name="b2s")
            nc.gpsimd.dma_start(
                b2s[:], b2_d.ap().rearrange("(c p) -> p c", p=P))

            # ---------------- PE warmup (HAM ramp) --------------------------
            wub = const.tile([P, 256], BF16, name="wub")
            nc.vector.memset(wub[:], 0.0)
            wuw = const.tile([P, P], BF16, name="wuw")
            nc.vector.memset(wuw[:], 0.0)
            ph = _Ph()
            pe_phases.append(ph)
            wups = mpsum.tile([P, NB32], F32, tag="mm", name="wups")
            for i in range(20):
                ph.add(nc.tensor.matmul(wups[:, 0:256], wuw[:], wub[:],
                                        start=(i == 0), stop=(i == 19)))

            # ---------------- initial LIF state ----------------------------
            mem1_cur = statep.tile([P, HC1, 32], F32, tag="mem1",
                                   name="mem1_0")
            nc.vector.memset(mem1_cur[:], 0.0)
            mem2_cur = statep.tile([P, HC2, 32], F32, tag="mem2",
                                   name="mem2_0")
            nc.vector.memset(mem2_cur[:], 0.0)
            spk2_fin = const.tile([P, HC2, 32], F32, name="spk2_fin")

            # ---------------- outputs helper --------------------------------
            def emit_out(state, nch, out_d):
                ph = _Ph()
                pe_phases.append(ph)
                for hc in range(nch):
                    ps = tpsum.tile([32, P], F32, tag="tp", name="ops")
                    ph.add(nc.tensor.transpose(ps[:], state[:, hc, :],
                                               ident[:]))
                    sb = outp.tile([32, P], F32, tag="osb", name="osb")
                    nc.scalar.activation(sb[:], ps[:], AF.Copy)
                    nc.sync.dma_start(
                        out_d.ap()[:, hc * P:(hc + 1) * P], sb[:])

            # ---------------- per-block emitters ----------------------------
            def x_and_mm1(nb):
                """xT load + matmul1 for block nb -> cur1_subs"""
                ph = _Ph()
                pe_phases.append(ph)
                t0 = nb * T_NB
                xt = xtp.tile([P, KC1, 2, NB32], F16, tag="xt", name="xt")
                # block 0 is latency-critical at startup: spread its 8 chunk
                # loads over two otherwise-idle queues (weights own sync/
                # scalar); later blocks prefetch leisurely on gpsimd.
                for kc in range(KC1):
                    xq = (nc.gpsimd if kc % 2 == 0 else nc.scalar) \
                        if nb == 0 else nc.gpsimd
                    xq.dma_start(
                        xt[:, kc, :, :],
                        xt_d.ap()[kc * P:(kc + 1) * P, :,
                                  t0 * 32:(t0 + T_NB) * 32])

                cur1_subs = [curp.tile([P, SUBG, HC1, 32], F32, tag="cur1",
                                       bufs=3, name="cur1")
                             for _ in range(NSUBG)]
                for hq in range(HC1 // HCQ):
                    pss = [mpsum.tile([P, NB32], F32, tag="mm", name="mm1ps")
                           for _ in range(HCQ)]
                    for kc in range(KC1):
                        w1tt = w1tp.tile([P, 2, HCQ * P], F16, tag="w1t",
                                         name="w1tt")
                        dq = nc.sync if kc % 2 == 0 else nc.scalar
                        dq.dma_start(
                            w1tt[:],
                            w1t_d.ap()[kc * P:(kc + 1) * P, :,
                                       hq * HCQ * P:(hq + 1) * HCQ * P])
                        rhs_h = xt[:, kc, 0, :]
                        rhs_l = xt[:, kc, 1, :]
                        for i in range(HCQ):
                            # W1h*xh + W1h*xl + W1l*xh (~22-bit effective);
                            # consecutive same-stationary passes share LDW.
                            ph.add(nc.tensor.matmul(
                                pss[i][:], w1tt[:, 0, i * P:(i + 1) * P],
                                rhs_h, start=(kc == 0), stop=False))
                            ph.add(nc.tensor.matmul(
                                pss[i][:], w1tt[:, 0, i * P:(i + 1) * P],
                                rhs_l, start=False, stop=False))
                            ph.add(nc.tensor.matmul(
                                pss[i][:], w1tt[:, 1, i * P:(i + 1) * P],
                                rhs_h, start=False, stop=(kc == KC1 - 1)))
                    for s in range(NSUBG):
                        for i in range(HCQ):
                            hc = hq * HCQ + i
                            psv = pss[i].rearrange("p (t b) -> p t b", b=32)
                            nc.scalar.activation(
                                cur1_subs[s][:, :, hc, :],
                                psv[:, s * SUBG:(s + 1) * SUBG, :],
                                AF.Identity, bias=b1s[:, hc:hc + 1],
                                scale=SC1)
                return cur1_subs

            # ---------------- scan emitters ---------------------------------
            def scan1(cur1_subs):
                """LIF-1 scan (T_NB steps) -> (fp16 {0,1} spikes,
                fp8 {0,2^-8} spikes) tiles, both [(kc,t,b)]."""
                nonlocal mem1_cur
                spk1 = spk1p.tile([P, HC1, NB32], F16, tag="spk1", bufs=2,
                                  name="spk1")
                spk8 = spk1p.tile([P, HC1, NB32], F8, tag="spk8", bufs=2,
                                  name="spk8")
                for tr in range(T_NB):
                    cur_t = cur1_subs[tr // SUBG][:, tr % SUBG]
                    negz = negzp.tile([P, HC1, 32], F32, tag="negz",
                                      name="negz")
                    nc.vector.scalar_tensor_tensor(
                        negz[:], mem1_cur[:], THR, cur_t,
                        ALU.is_gt, ALU.subtract)
                    mem1_new = statep.tile([P, HC1, 32], F32, tag="mem1",
                                           name="mem1")
                    nc.vector.scalar_tensor_tensor(
                        mem1_new[:], mem1_cur[:], BETA, negz[:],
                        ALU.mult, ALU.subtract)
                    mem1_cur = mem1_new
                    # spike of step t thresholds the POST-update membrane
                    nc.vector.tensor_scalar(
                        spk1[:, :, tr * 32:(tr + 1) * 32], mem1_cur[:],
                        THR, None, ALU.is_gt)
                    nc.vector.tensor_scalar(
                        spk8[:, :, tr * 32:(tr + 1) * 32], mem1_cur[:],
                        THR, 1.0 / 256.0, ALU.is_gt, ALU.mult)
                return spk1, spk8

            def mm2(spks):
                """cur2[(t,mc,b)] = W2 @ spk1^T + b2: one fp16 hi pass +
                one fp8 DoubleRow residual pass (K=256/inst), same PSUM."""
                spk1, spk8 = spks
                ph = _Ph()
                pe_phases.append(ph)
                cur2_subs = [curp.tile([P, SUBG, HC2, 32], F32, tag="cur2",
                                       bufs=2, name="cur2")
                             for _ in range(NSUBG)]
                KP = HC1 // 2
                for mq in range(HC2 // MCQ):
                    pss = [mpsum.tile([P, NB32], F32, tag="mm",
                                      name="mm2ps")
                           for _ in range(MCQ)]
                    for kc in range(HC1):
                        wt = w2tp.tile([P, MCQ * P], F16, tag="w2t",
                                       name="w2t")
                        dq = nc.sync if kc % 2 == 0 else nc.scalar
                        dq.dma_start(
                            wt[:],
                            w2t_d.ap()[kc * P:(kc + 1) * P,
                                       mq * MCQ * P:(mq + 1) * MCQ * P])
                        rhs = spk1[:, kc, :]
                        for i in range(MCQ):
                            ph.add(nc.tensor.matmul(
                                pss[i][:], wt[:, i * P:(i + 1) * P], rhs,
                                start=(kc == 0), stop=False))
                    for q in range(KP):
                        wl = w2lp.tile([P, 2, MCQ * P], F8, tag="w2l",
                                       name="w2l")
                        nc.gpsimd.dma_start(
                            wl[:],
                            w2l_d.ap()[q * P:(q + 1) * P, :,
                                       mq * MCQ * P:(mq + 1) * MCQ * P])
                        rhs = spk8[:, 2 * q:2 * q + 2, :]
                        for i in range(MCQ):
                            ph.add(nc.tensor.matmul(
                                pss[i][:], wl[:, :, i * P:(i + 1) * P], rhs,
                                start=False, stop=(q == KP - 1),
                                perf_mode=mybir.MatmulPerfMode.DoubleRow))
                    # sub-major evac order so scan2 step 0's inputs (s=0 of
                    # every mc) complete as early as possible
                    for s in range(NSUBG):
                        for i in range(MCQ):
                            mc = mq * MCQ + i
                            psv = pss[i].rearrange("p (t b) -> p t b", b=32)
                            nc.scalar.activation(
                                cur2_subs[s][:, :, mc, :],
                                psv[:, s * SUBG:(s + 1) * SUBG, :],
                                AF.Identity, bias=b2s[:, mc:mc + 1],
                                scale=SC2)
                return cur2_subs

            def scan2(cur2_subs, nb):
                nonlocal mem2_cur
                t0 = nb * T_NB
                for tr in range(T_NB):
                    t = t0 + tr
                    cur_t = cur2_subs[tr // SUBG][:, tr % SUBG]
                    negz = negzp.tile([P, HC2, 32], F32, tag="negz",
                                      name="negz")
                    nc.vector.scalar_tensor_tensor(
                        negz[:], mem2_cur[:], THR, cur_t,
                        ALU.is_gt, ALU.subtract)
                    mem2_new = statep.tile([P, HC2, 32], F32, tag="mem2",
                                           name="mem2")
                    nc.vector.scalar_tensor_tensor(
                        mem2_new[:], mem2_cur[:], BETA, negz[:],
                        ALU.mult, ALU.subtract)
                    mem2_cur = mem2_new
                    if t == T - 1:
                        nc.vector.tensor_scalar(
                            spk2_fin[:], mem2_cur[:], THR, None, ALU.is_gt)

            # ---------------- main t-block pipeline -------------------------
            # Software pipelining, two levels:
            #  - PE stream: mm1(nb+1) is emitted BEFORE mm2(nb) so the PE
            #    fills the scan1 latency instead of stalling on spk1.
            #  - DVE stream: scan1(nb+1) is emitted BEFORE scan2(nb) so the
            #    (FIFO) vector engine runs scan1(nb+1) during mm2(nb) instead
            #    of queuing it behind scan2(nb), which can only start once
            #    mm2(nb) is nearly done. This keeps spk1(nb+1) ready the
            #    moment mm2(nb) retires -- critical for the last block, where
            #    no mm1(nb+1) exists to hide the wait.
            cur1_next = x_and_mm1(0)
            spk1_next = scan1(cur1_next)
            for nb in range(NNB):
                spk1_cur = spk1_next
                if nb + 1 < NNB:
                    cur1_next = x_and_mm1(nb + 1)
                if nb == NNB - 1:
                    emit_out(mem1_cur, HC1, mem1_d)
                cur2_subs = mm2(spk1_cur)
                if nb + 1 < NNB:
                    spk1_next = scan1(cur1_next)
                scan2(cur2_subs, nb)

            # ---------------- remaining outputs -----------------------------
            emit_out(mem2_cur, HC2, mem2_d)
            emit_out(spk2_fin, HC2, spk2_d)

            # chain consecutive PE phases: every inst of phase b ordered
            # after the last inst of phase a (order-only deps)
            for a, b in zip(pe_phases, pe_phases[1:]):
                if a.insts and b.insts:
                    for bi in b.insts:
                        add_dep_helper(bi, a.insts[-1], sync=False,
                                       reason="PE phase ordering")

    nc.compile()
    return nc


_NC_CACHE = {}


def _get_nc():
    if "full" not in _NC_CACHE:
        _NC_CACHE["full"] = build_snn()
    return _NC_CACHE["full"]


def _dekker_f16(a):
    """Split fp32 array into fp16 hi+lo terms stacked on axis 1."""
    hi = a.astype(np.float16)
    lo = (a - hi.astype(np.float32)).astype(np.float16)
    return np.ascontiguousarray(np.stack([hi, lo], axis=1))


def prep_inputs(x, W1, b1, W2, b2):
    """Host-side prep: shard x over cores (transposed to [d, (t,b)]) and
    Dekker-split x, W1, W2 into pre-scaled fp16 hi+lo pairs."""
    x = np.asarray(x, np.float32)
    W1 = np.asarray(W1, np.float32)
    b1 = np.ascontiguousarray(np.asarray(b1, np.float32))
    W2 = np.asarray(W2, np.float32)
    b2 = np.ascontiguousarray(np.asarray(b2, np.float32))
    B, T, D = x.shape

    W1Thl = _dekker_f16(W1.T * np.float32(S_W1))        # [D, 2, H1]

    # W2 -> fp16 hi + fp8 residual (scaled 2^8) in DoubleRow pair layout
    H1 = W2.shape[0]
    W2s = np.ascontiguousarray(W2.T) * np.float32(S_W2)  # [H1, H2]
    W2Th = W2s.astype(np.float16)
    r = (W2s - W2Th.astype(np.float32)) * np.float32(256.0)
    W2l8 = np.ascontiguousarray(
        r.astype(ml_dtypes.float8_e4m3)
        .reshape(H1 // 256, 2, 128, -1)
        .transpose(0, 2, 1, 3)
        .reshape(H1 // 2, 2, -1))                       # [(q p), j, H2]

    bl = B // N_CORES
    in_maps = []
    for c in range(N_CORES):
        xc = x[c * bl:(c + 1) * bl]                     # [bl, T, D]
        xT = xc.transpose(2, 1, 0).reshape(D, T * bl)   # [d, (t,b)] t-major
        xThl = _dekker_f16(xT * np.float32(S_X))        # [D, 2, (t,b)]
        in_maps.append({
            "xThl": xThl, "W1Thl": W1Thl, "b1": b1, "W2Th": W2Th,
            "W2l8": W2l8, "b2": b2,
        })
    return in_maps


def kernel(x, W1, b1, W2, b2):
    """Full-input entry point: shards B across 8 NeuronCores, returns full
    (spk2, mem1, mem2) exactly like reference()."""
    nc = _get_nc()
    in_maps = prep_inputs(x, W1, b1, W2, b2)
    res = run_bass_kernel_spmd(nc, in_maps, core_ids=list(range(N_CORES)))
    spk2 = np.concatenate([res.results[c]["spk2"] for c in range(N_CORES)], 0)
    mem1 = np.concatenate([res.results[c]["mem1"] for c in range(N_CORES)], 0)
    mem2 = np.concatenate([res.results[c]["mem2"] for c in range(N_CORES)], 0)
    return spk2, mem1, mem2



# revision 44
# speedup vs baseline: 1.2375x; 1.0325x over previous
"""Feedforward SNN (Linear -> LIF) x2 kernel for Trainium2, 8-core data parallel.

Per-core plan (B sharded 8 ways, BL=32 samples/core):
  - Host pre-transposes operands once (cheap numpy) and splits every matmul
    operand into low-precision terms (pre-scaled by powers of 2 to dodge
    subnormals). fp16 runs the PE at 1.0 cycles/row (vs fp32's 4.0) and its
    11-bit mantissa is held exactly by the PE's internal FP22 format, so an
    fp16 Dekker pair carries ~22 significant bits -- fp32-grade here.
  - Layer-1 currents for ALL timesteps: Cur1[h1, (t,b)] = W1 @ x^T as THREE
    fp16 matmuls (W1h*xh + W1l*xh + W1h*xl; the dropped W1l*xl term is
    ~2^-22 relative). The chaotic LIF-1 threshold dynamics NEED this much
    precision: any scheme leaving x or W1 at <=16 effective bits flips
    enough layer-1 spikes to blow up layer-2 error (emulated + HW-checked).
  - Layer-2 currents: W2 = fp16 hi (11 bits) + ONE fp8(e4m3) residual term
    (~5 more bits) consumed by a DoubleRow matmul: fp8 pairs two K-chunks
    per instruction (measured 2x K-throughput), so the residual pass costs
    half a regular pass. The residual is scaled 2^8 into e4m3's range and
    its spike tensor is valued {0, 2^-8} (exact in e4m3), so both passes'
    products land at the same scale and share one PSUM accumulation.
    W2 at ~16 bits is enough: cur2 errors are unbiased and layer-2 spikes
    see them only once (measured end-to-end 6.4e-3 vs the 2e-2 gate,
    reproducing the host fp8-emulation exactly).
  - LIF scans over t on [128, HC*32] tiles (partition = h % 128, free =
    (hchunk, b)); fused scalar_tensor_tensor DVE ops; scan-1 additionally
    materializes the fp16 {0,1} and fp8 {0,2^-8} spike tensors.
  - The pre-scales are undone for free in the PSUM->SBUF bias-add
    activation (out = psum*scale + bias), batched 8 timesteps per call.
  - Software-pipelined, two levels: mm1(nb+1) is emitted before mm2(nb) so
    the PE fills the scan1 latency, and scan1(nb+1) is emitted before
    scan2(nb) so the FIFO vector engine runs scan1(nb+1) during mm2(nb) --
    without this the last block exposes a ~40us serial tail.
  - Outputs leave via PE transposes batched 4 h-chunks per PSUM tile ->
    one ACT copy -> one contiguous DMA.
"""

import os
import sys

import numpy as np

for _p in ("/opt/trn_rl_repo", "/root/.axon_site/_ro/trn_rl_repo"):
    if os.path.isdir(_p) and _p not in sys.path:
        sys.path.insert(0, _p)

import ml_dtypes  # noqa: E402

import concourse.bass as bass  # noqa: E402
import concourse.mybir as mybir  # noqa: E402
import concourse.tile as tile  # noqa: E402
from concourse import bacc  # noqa: E402
from concourse.bass_utils import run_bass_kernel_spmd  # noqa: E402
from concourse.masks import make_identity  # noqa: E402
from concourse.tile_rust import add_dep_helper  # noqa: E402

F32 = mybir.dt.float32
F32R = mybir.dt.float32r
BF16 = mybir.dt.bfloat16
F16 = mybir.dt.float16
F8 = mybir.dt.float8e4
ALU = mybir.AluOpType
AF = mybir.ActivationFunctionType

BETA = 0.9
THR = 1.0

# fp16 Dekker-split pre-scales (powers of 2; exact in fp32) and the evac
# scales that undo them during the PSUM->SBUF bias-add.
S_X = 16.0
S_W1 = 256.0
S_W2 = 256.0
SC1 = 1.0 / (S_X * S_W1)
SC2 = 1.0 / S_W2

B_FULL, T_FULL, D_FULL, H1_FULL, H2_FULL = 256, 64, 1024, 2048, 2048
N_CORES = 8
BL = B_FULL // N_CORES  # 32


def build_snn(T=T_FULL, D=D_FULL, H1=H1_FULL, H2=H2_FULL, T_NB=16):
    """Build the single-core Bass program (identical across the 8 cores)."""
    P = 128
    KC1 = D // P
    HC1 = H1 // P
    HC2 = H2 // P
    NNB = T // T_NB
    SUB = min(4, T_NB)
    NSUB = T_NB // SUB
    SUBG = min(8, T_NB)
    NSUBG = T_NB // SUBG
    MCQ = min(4, HC2)
    HCQ = min(4, HC1)
    NB32 = T_NB * 32          # matmul free dim per t-block

    assert T % T_NB == 0 and T_NB % SUB == 0
    assert HC2 % MCQ == 0 and HC1 % HCQ == 0

    nc = bacc.Bacc("TRN2", target_bir_lowering=False, debug=False)

    xt_d = nc.dram_tensor("xThl", [D, 2, T * BL], F16, kind="ExternalInput")
    w1t_d = nc.dram_tensor("W1Thl", [D, 2, H1], F16, kind="ExternalInput")
    b1_d = nc.dram_tensor("b1", [H1], F32, kind="ExternalInput")
    # W2 = fp16 hi (11 bits) + one fp8 residual term consumed by a DoubleRow
    # matmul (2x K per instruction).  W2l8 is in DR pair layout: row
    # (q*128+p) holds the pair h1=(2q+j)*128+p along j, scaled 2^8 so the
    # residual sits in e4m3's normal range; the matching spike tensor is
    # {0, 2^-8} so the products land at the same scale as the hi pass and
    # share its PSUM accumulation.
    w2t_d = nc.dram_tensor("W2Th", [H1, H2], F16, kind="ExternalInput")
    w2l_d = nc.dram_tensor("W2l8", [H1 // 2, 2, H2], F8,
                           kind="ExternalInput")
    b2_d = nc.dram_tensor("b2", [H2], F32, kind="ExternalInput")

    spk2_d = nc.dram_tensor("spk2", [BL, H2], F32, kind="ExternalOutput")
    mem1_d = nc.dram_tensor("mem1", [BL, H1], F32, kind="ExternalOutput")
    mem2_d = nc.dram_tensor("mem2", [BL, H2], F32, kind="ExternalOutput")

    with tile.TileContext(nc) as tc:
        from contextlib import ExitStack
        ctx = ExitStack()
        with ctx:
            const = ctx.enter_context(tc.tile_pool(name="const", bufs=1))
            xtp = ctx.enter_context(tc.tile_pool(name="xtp", bufs=2))
            w1tp = ctx.enter_context(tc.tile_pool(name="w1tp", bufs=7))
            w2tp = ctx.enter_context(tc.tile_pool(name="w2tp", bufs=8))
            w2lp = ctx.enter_context(tc.tile_pool(name="w2lp", bufs=6))
            curp = ctx.enter_context(tc.tile_pool(name="curp", bufs=6))
            spk1p = ctx.enter_context(tc.tile_pool(name="spk1p", bufs=1))
            statep = ctx.enter_context(tc.tile_pool(name="statep", bufs=2))
            negzp = ctx.enter_context(tc.tile_pool(name="negzp", bufs=1))
            outp = ctx.enter_context(tc.tile_pool(name="outp", bufs=2))
            tpsum = ctx.enter_context(
                tc.tile_pool(name="tpsum", bufs=2, space="PSUM"))
            mpsum = ctx.enter_context(
                tc.tile_pool(name="mpsum", bufs=6, space="PSUM"))

            ident = const.tile([P, P], F32, name="ident")
            make_identity(nc, ident)

            # PE phase chaining (order-only deps): keeps fp32-mm, bf16-mm
            # and transpose phases from interleaving in the PE stream.
            pe_phases = []

            class _Ph:
                def __init__(self):
                    self.insts = []

                def add(self, bi):
                    self.insts.append(bi.ins)

            b1s = const.tile([P, HC1], F32, name="b1s")
            nc.gpsimd.dma_start(
                b1s[:], b1_d.ap().rearrange("(c p) -> p c", p=P))
            b2s = const.tile([P, HC2], F32, name="b2s")
            nc.gpsimd.dma_start(
                b2s[:], b2_d.ap().rearrange("(c p) -> p c", p=P))

            # ---------------- PE warmup (HAM ramp) --------------------------
            wub = const.tile([P, 256], BF16, name="wub")
            nc.vector.memset(wub[:], 0.0)
            wuw = const.tile([P, P], BF16, name="wuw")
            nc.vector.memset(wuw[:], 0.0)
            ph = _Ph()
            pe_phases.append(ph)
            wups = mpsum.tile([P, NB32], F32, tag="mm", name="wups")
            for i in range(20):
                ph.add(nc.tensor.matmul(wups[:, 0:256], wuw[:], wub[:],
                                        start=(i == 0), stop=(i == 19)))

            # ---------------- initial LIF state ----------------------------
            mem1_cur = statep.tile([P, HC1, 32], F32, tag="mem1",
                                   name="mem1_0")
            nc.vector.memset(mem1_cur[:], 0.0)
            mem2_cur = statep.tile([P, HC2, 32], F32, tag="mem2",
                                   name="mem2_0")
            nc.vector.memset(mem2_cur[:], 0.0)
            spk2_fin = const.tile([P, HC2, 32], F32, name="spk2_fin")

            # ---------------- outputs helper --------------------------------
            def emit_out(state, nch, out_d):
                ph = _Ph()
                pe_phases.append(ph)
                for g in range(nch // 4):
                    ps = tpsum.tile([32, 4 * P], F32, tag="tp", name="ops")
                    for j in range(4):
                        ph.add(nc.tensor.transpose(
                            ps[:, j * P:(j + 1) * P],
                            state[:, g * 4 + j, :], ident[:]))
                    sb = outp.tile([32, 4 * P], F32, tag="osb", name="osb")
                    nc.scalar.activation(sb[:], ps[:], AF.Copy)
                    nc.sync.dma_start(
                        out_d.ap()[:, g * 4 * P:(g + 1) * 4 * P], sb[:])

            # ---------------- per-block emitters ----------------------------
            def x_and_mm1(nb):
                """xT load + matmul1 for block nb -> cur1_subs"""
                ph = _Ph()
                pe_phases.append(ph)
                t0 = nb * T_NB
                xt = xtp.tile([P, KC1, 2, NB32], F16, tag="xt", name="xt")
                # block 0 is latency-critical at startup: spread its 8 chunk
                # loads over two otherwise-idle queues (weights own sync/
                # scalar); later blocks prefetch leisurely on gpsimd.
                for kc in range(KC1):
                    xq = nc.gpsimd
                    xq.dma_start(
                        xt[:, kc, :, :],
                        xt_d.ap()[kc * P:(kc + 1) * P, :,
                                  t0 * 32:(t0 + T_NB) * 32])

                cur1_subs = [curp.tile([P, SUBG, HC1, 32], F32, tag="cur1",
                                       bufs=3, name="cur1")
                             for _ in range(NSUBG)]
                for hq in range(HC1 // HCQ):
                    pss = [mpsum.tile([P, NB32], F32, tag="mm", name="mm1ps")
                           for _ in range(HCQ)]
                    for kc in range(KC1):
                        w1tt = w1tp.tile([P, 2, HCQ * P], F16, tag="w1t",
                                         name="w1tt")
                        dq = nc.sync if kc % 2 == 0 else nc.scalar
                        dq.dma_start(
                            w1tt[:],
                            w1t_d.ap()[kc * P:(kc + 1) * P, :,
                                       hq * HCQ * P:(hq + 1) * HCQ * P])
                        rhs_h = xt[:, kc, 0, :]
                        rhs_l = xt[:, kc, 1, :]
                        for i in range(HCQ):
                            # W1h*xh + W1h*xl + W1l*xh (~22-bit effective);
                            # consecutive same-stationary passes share LDW.
                            ph.add(nc.tensor.matmul(
                                pss[i][:], w1tt[:, 0, i * P:(i + 1) * P],
                                rhs_h, start=(kc == 0), stop=False))
                            ph.add(nc.tensor.matmul(
                                pss[i][:], w1tt[:, 0, i * P:(i + 1) * P],
                                rhs_l, start=False, stop=False))
                            ph.add(nc.tensor.matmul(
                                pss[i][:], w1tt[:, 1, i * P:(i + 1) * P],
                                rhs_h, start=False, stop=(kc == KC1 - 1)))
                    for s in range(NSUBG):
                        for i in range(HCQ):
                            hc = hq * HCQ + i
                            psv = pss[i].rearrange("p (t b) -> p t b", b=32)
                            nc.scalar.activation(
                                cur1_subs[s][:, :, hc, :],
                                psv[:, s * SUBG:(s + 1) * SUBG, :],
                                AF.Identity, bias=b1s[:, hc:hc + 1],
                                scale=SC1)
                return cur1_subs

            # ---------------- scan emitters ---------------------------------
            def scan1(cur1_subs):
                """LIF-1 scan (T_NB steps) -> (fp16 {0,1} spikes,
                fp8 {0,2^-8} spikes) tiles, both [(kc,t,b)]."""
                nonlocal mem1_cur
                spk1 = spk1p.tile([P, HC1, NB32], F16, tag="spk1", bufs=2,
                                  name="spk1")
                spk8 = spk1p.tile([P, HC1, NB32], F8, tag="spk8", bufs=2,
                                  name="spk8")
                for tr in range(T_NB):
                    cur_t = cur1_subs[tr // SUBG][:, tr % SUBG]
                    negz = negzp.tile([P, HC1, 32], F32, tag="negz",
                                      name="negz")
                    nc.vector.scalar_tensor_tensor(
                        negz[:], mem1_cur[:], THR, cur_t,
                        ALU.is_gt, ALU.subtract)
                    mem1_new = statep.tile([P, HC1, 32], F32, tag="mem1",
                                           name="mem1")
                    nc.vector.scalar_tensor_tensor(
                        mem1_new[:], mem1_cur[:], BETA, negz[:],
                        ALU.mult, ALU.subtract)
                    mem1_cur = mem1_new
                    # spike of step t thresholds the POST-update membrane
                    nc.vector.tensor_scalar(
                        spk1[:, :, tr * 32:(tr + 1) * 32], mem1_cur[:],
                        THR, None, ALU.is_gt)
                    nc.vector.tensor_scalar(
                        spk8[:, :, tr * 32:(tr + 1) * 32], mem1_cur[:],
                        THR, 1.0 / 256.0, ALU.is_gt, ALU.mult)
                return spk1, spk8

            def mm2(spks):
                """cur2[(t,mc,b)] = W2 @ spk1^T + b2: one fp16 hi pass +
                one fp8 DoubleRow residual pass (K=256/inst), same PSUM."""
                spk1, spk8 = spks
                ph = _Ph()
                pe_phases.append(ph)
                cur2_subs = [curp.tile([P, SUBG, HC2, 32], F32, tag="cur2",
                                       bufs=2, name="cur2")
                             for _ in range(NSUBG)]
                KP = HC1 // 2
                for mq in range(HC2 // MCQ):
                    pss = [mpsum.tile([P, NB32], F32, tag="mm",
                                      name="mm2ps")
                           for _ in range(MCQ)]
                    for kc in range(HC1):
                        wt = w2tp.tile([P, MCQ * P], F16, tag="w2t",
                                       name="w2t")
                        dq = nc.sync if kc % 2 == 0 else nc.scalar
                        dq.dma_start(
                            wt[:],
                            w2t_d.ap()[kc * P:(kc + 1) * P,
                                       mq * MCQ * P:(mq + 1) * MCQ * P])
                        rhs = spk1[:, kc, :]
                        for i in range(MCQ):
                            ph.add(nc.tensor.matmul(
                                pss[i][:], wt[:, i * P:(i + 1) * P], rhs,
                                start=(kc == 0), stop=False))
                    for q in range(KP):
                        wl = w2lp.tile([P, 2, MCQ * P], F8, tag="w2l",
                                       name="w2l")
                        nc.gpsimd.dma_start(
                            wl[:],
                            w2l_d.ap()[q * P:(q + 1) * P, :,
                                       mq * MCQ * P:(mq + 1) * MCQ * P])
                        rhs = spk8[:, 2 * q:2 * q + 2, :]
                        for i in range(MCQ):
                            ph.add(nc.tensor.matmul(
                                pss[i][:], wl[:, :, i * P:(i + 1) * P], rhs,
                                start=False, stop=(q == KP - 1),
                                perf_mode=mybir.MatmulPerfMode.DoubleRow))
                    # sub-major evac order so scan2 step 0's inputs (s=0 of
                    # every mc) complete as early as possible
                    for s in range(NSUBG):
                        for i in range(MCQ):
                            mc = mq * MCQ + i
                            psv = pss[i].rearrange("p (t b) -> p t b", b=32)
                            nc.scalar.activation(
                                cur2_subs[s][:, :, mc, :],
                                psv[:, s * SUBG:(s + 1) * SUBG, :],
                                AF.Identity, bias=b2s[:, mc:mc + 1],
                                scale=SC2)
                return cur2_subs

            def scan2(cur2_subs, nb):
                nonlocal mem2_cur
                t0 = nb * T_NB
                for tr in range(T_NB):
                    t = t0 + tr
                    cur_t = cur2_subs[tr // SUBG][:, tr % SUBG]
                    negz = negzp.tile([P, HC2, 32], F32, tag="negz",
                                      name="negz")
                    nc.vector.scalar_tensor_tensor(
                        negz[:], mem2_cur[:], THR, cur_t,
                        ALU.is_gt, ALU.subtract)
                    mem2_new = statep.tile([P, HC2, 32], F32, tag="mem2",
                                           name="mem2")
                    nc.vector.scalar_tensor_tensor(
                        mem2_new[:], mem2_cur[:], BETA, negz[:],
                        ALU.mult, ALU.subtract)
                    mem2_cur = mem2_new
                    if t == T - 1:
                        nc.vector.tensor_scalar(
                            spk2_fin[:], mem2_cur[:], THR, None, ALU.is_gt)

            # ---------------- main t-block pipeline -------------------------
            # Software pipelining, two levels:
            #  - PE stream: mm1(nb+1) is emitted BEFORE mm2(nb) so the PE
            #    fills the scan1 latency instead of stalling on spk1.
            #  - DVE stream: scan1(nb+1) is emitted BEFORE scan2(nb) so the
            #    (FIFO) vector engine runs scan1(nb+1) during mm2(nb) instead
            #    of queuing it behind scan2(nb), which can only start once
            #    mm2(nb) is nearly done. This keeps spk1(nb+1) ready the
            #    moment mm2(nb) retires -- critical for the last block, where
            #    no mm1(nb+1) exists to hide the wait.
            cur1_next = x_and_mm1(0)
            spk1_next = scan1(cur1_next)
            for nb in range(NNB):
                spk1_cur = spk1_next
                if nb + 1 < NNB:
                    cur1_next = x_and_mm1(nb + 1)
                if nb == NNB - 1:
                    emit_out(mem1_cur, HC1, mem1_d)
                cur2_subs = mm2(spk1_cur)
                if nb + 1 < NNB:
                    spk1_next = scan1(cur1_next)
                scan2(cur2_subs, nb)

            # ---------------- remaining outputs -----------------------------
            emit_out(mem2_cur, HC2, mem2_d)
            emit_out(spk2_fin, HC2, spk2_d)

            # chain consecutive PE phases: every inst of phase b ordered
            # after the last inst of phase a (order-only deps)
            for a, b in zip(pe_phases, pe_phases[1:]):
                if a.insts and b.insts:
                    for bi in b.insts:
                        add_dep_helper(bi, a.insts[-1], sync=False,
                                       reason="PE phase ordering")

    nc.compile()
    return nc


_NC_CACHE = {}


def _get_nc():
    if "full" not in _NC_CACHE:
        _NC_CACHE["full"] = build_snn()
    return _NC_CACHE["full"]


def _dekker_f16(a):
    """Split fp32 array into fp16 hi+lo terms stacked on axis 1."""
    hi = a.astype(np.float16)
    lo = (a - hi.astype(np.float32)).astype(np.float16)
    return np.ascontiguousarray(np.stack([hi, lo], axis=1))


def prep_inputs(x, W1, b1, W2, b2):
    """Host-side prep: shard x over cores (transposed to [d, (t,b)]) and
    Dekker-split x, W1, W2 into pre-scaled fp16 hi+lo pairs."""
    x = np.asarray(x, np.float32)
    W1 = np.asarray(W1, np.float32)
    b1 = np.ascontiguousarray(np.asarray(b1, np.float32))
    W2 = np.asarray(W2, np.float32)
    b2 = np.ascontiguousarray(np.asarray(b2, np.float32))
    B, T, D = x.shape

    W1Thl = _dekker_f16(W1.T * np.float32(S_W1))        # [D, 2, H1]

    # W2 -> fp16 hi + fp8 residual (scaled 2^8) in DoubleRow pair layout
    H1 = W2.shape[0]
    W2s = np.ascontiguousarray(W2.T) * np.float32(S_W2)  # [H1, H2]
    W2Th = W2s.astype(np.float16)
    r = (W2s - W2Th.astype(np.float32)) * np.float32(256.0)
    W2l8 = np.ascontiguousarray(
        r.astype(ml_dtypes.float8_e4m3)
        .reshape(H1 // 256, 2, 128, -1)
        .transpose(0, 2, 1, 3)
        .reshape(H1 // 2, 2, -1))                       # [(q p), j, H2]

    bl = B // N_CORES
    in_maps = []
    for c in range(N_CORES):
        xc = x[c * bl:(c + 1) * bl]                     # [bl, T, D]
        xT = xc.transpose(2, 1, 0).reshape(D, T * bl)   # [d, (t,b)] t-major
        xThl = _dekker_f16(xT * np.float32(S_X))        # [D, 2, (t,b)]
        in_maps.append({
            "xThl": xThl, "W1Thl": W1Thl, "b1": b1, "W2Th": W2Th,
            "W2l8": W2l8, "b2": b2,
        })
    return in_maps


def kernel(x, W1, b1, W2, b2):
    """Full-input entry point: shards B across 8 NeuronCores, returns full
    (spk2, mem1, mem2) exactly like reference()."""
    nc = _get_nc()
    in_maps = prep_inputs(x, W1, b1, W2, b2)
    res = run_bass_kernel_spmd(nc, in_maps, core_ids=list(range(N_CORES)))
    spk2 = np.concatenate([res.results[c]["spk2"] for c in range(N_CORES)], 0)
    mem1 = np.concatenate([res.results[c]["mem1"] for c in range(N_CORES)], 0)
    mem2 = np.concatenate([res.results[c]["mem2"] for c in range(N_CORES)], 0)
    return spk2, mem1, mem2



# revision 45
# speedup vs baseline: 1.2399x; 1.0019x over previous
"""Feedforward SNN (Linear -> LIF) x2 kernel for Trainium2, 8-core data parallel.

Per-core plan (B sharded 8 ways, BL=32 samples/core):
  - Host pre-transposes operands once (cheap numpy) and splits every matmul
    operand into low-precision terms (pre-scaled by powers of 2 to dodge
    subnormals). fp16 runs the PE at 1.0 cycles/row (vs fp32's 4.0) and its
    11-bit mantissa is held exactly by the PE's internal FP22 format, so an
    fp16 Dekker pair carries ~22 significant bits -- fp32-grade here.
  - Layer-1 currents for ALL timesteps: Cur1[h1, (t,b)] = W1 @ x^T as THREE
    fp16 matmuls (W1h*xh + W1l*xh + W1h*xl; the dropped W1l*xl term is
    ~2^-22 relative). The chaotic LIF-1 threshold dynamics NEED this much
    precision: any scheme leaving x or W1 at <=16 effective bits flips
    enough layer-1 spikes to blow up layer-2 error (emulated + HW-checked).
  - Layer-2 currents: W2 = fp16 hi (11 bits) + ONE fp8(e4m3) residual term
    (~5 more bits) consumed by a DoubleRow matmul: fp8 pairs two K-chunks
    per instruction (measured 2x K-throughput), so the residual pass costs
    half a regular pass. The residual is scaled 2^8 into e4m3's range and
    its spike tensor is valued {0, 2^-8} (exact in e4m3), so both passes'
    products land at the same scale and share one PSUM accumulation.
    W2 at ~16 bits is enough: cur2 errors are unbiased and layer-2 spikes
    see them only once (measured end-to-end 6.4e-3 vs the 2e-2 gate,
    reproducing the host fp8-emulation exactly).
  - LIF scans over t on [128, HC*32] tiles (partition = h % 128, free =
    (hchunk, b)); fused scalar_tensor_tensor DVE ops; scan-1 additionally
    materializes the fp16 {0,1} and fp8 {0,2^-8} spike tensors.
  - The pre-scales are undone for free in the PSUM->SBUF bias-add
    activation (out = psum*scale + bias), batched 8 timesteps per call.
  - Software-pipelined, two levels: mm1(nb+1) is emitted before mm2(nb) so
    the PE fills the scan1 latency, and scan1(nb+1) is emitted before
    scan2(nb) so the FIFO vector engine runs scan1(nb+1) during mm2(nb) --
    without this the last block exposes a ~40us serial tail.
  - Outputs leave via PE transposes batched 4 h-chunks per PSUM tile ->
    one ACT copy -> one contiguous DMA.
"""

import os
import sys

import numpy as np

for _p in ("/opt/trn_rl_repo", "/root/.axon_site/_ro/trn_rl_repo"):
    if os.path.isdir(_p) and _p not in sys.path:
        sys.path.insert(0, _p)

import ml_dtypes  # noqa: E402

import concourse.bass as bass  # noqa: E402
import concourse.mybir as mybir  # noqa: E402
import concourse.tile as tile  # noqa: E402
from concourse import bacc  # noqa: E402
from concourse.bass_utils import run_bass_kernel_spmd  # noqa: E402
from concourse.masks import make_identity  # noqa: E402
from concourse.tile_rust import add_dep_helper  # noqa: E402

F32 = mybir.dt.float32
F32R = mybir.dt.float32r
BF16 = mybir.dt.bfloat16
F16 = mybir.dt.float16
F8 = mybir.dt.float8e4
ALU = mybir.AluOpType
AF = mybir.ActivationFunctionType

BETA = 0.9
THR = 1.0

# fp16 Dekker-split pre-scales (powers of 2; exact in fp32) and the evac
# scales that undo them during the PSUM->SBUF bias-add.
S_X = 16.0
S_W1 = 256.0
S_W2 = 256.0
SC1 = 1.0 / (S_X * S_W1)
SC2 = 1.0 / S_W2

B_FULL, T_FULL, D_FULL, H1_FULL, H2_FULL = 256, 64, 1024, 2048, 2048
N_CORES = 8
BL = B_FULL // N_CORES  # 32


def build_snn(T=T_FULL, D=D_FULL, H1=H1_FULL, H2=H2_FULL, T_NB=16):
    """Build the single-core Bass program (identical across the 8 cores)."""
    P = 128
    KC1 = D // P
    HC1 = H1 // P
    HC2 = H2 // P
    NNB = T // T_NB
    SUB = min(4, T_NB)
    NSUB = T_NB // SUB
    SUBG = min(8, T_NB)
    NSUBG = T_NB // SUBG
    MCQ = min(4, HC2)
    HCQ = min(4, HC1)
    NB32 = T_NB * 32          # matmul free dim per t-block

    assert T % T_NB == 0 and T_NB % SUB == 0
    assert HC2 % MCQ == 0 and HC1 % HCQ == 0

    nc = bacc.Bacc("TRN2", target_bir_lowering=False, debug=False)

    xt_d = nc.dram_tensor("xThl", [D, 2, T * BL], F16, kind="ExternalInput")
    w1t_d = nc.dram_tensor("W1Thl", [D, 2, H1], F16, kind="ExternalInput")
    b1_d = nc.dram_tensor("b1", [H1], F32, kind="ExternalInput")
    # W2 = fp16 hi (11 bits) + one fp8 residual term consumed by a DoubleRow
    # matmul (2x K per instruction).  W2l8 is in DR pair layout: row
    # (q*128+p) holds the pair h1=(2q+j)*128+p along j, scaled 2^8 so the
    # residual sits in e4m3's normal range; the matching spike tensor is
    # {0, 2^-8} so the products land at the same scale as the hi pass and
    # share its PSUM accumulation.
    w2t_d = nc.dram_tensor("W2Th", [H1, H2], F16, kind="ExternalInput")
    w2l_d = nc.dram_tensor("W2l8", [H1 // 2, 2, H2], F8,
                           kind="ExternalInput")
    b2_d = nc.dram_tensor("b2", [H2], F32, kind="ExternalInput")

    spk2_d = nc.dram_tensor("spk2", [BL, H2], F32, kind="ExternalOutput")
    mem1_d = nc.dram_tensor("mem1", [BL, H1], F32, kind="ExternalOutput")
    mem2_d = nc.dram_tensor("mem2", [BL, H2], F32, kind="ExternalOutput")

    with tile.TileContext(nc) as tc:
        from contextlib import ExitStack
        ctx = ExitStack()
        with ctx:
            const = ctx.enter_context(tc.tile_pool(name="const", bufs=1))
            xtp = ctx.enter_context(tc.tile_pool(name="xtp", bufs=2))
            w1tp = ctx.enter_context(tc.tile_pool(name="w1tp", bufs=7))
            w2tp = ctx.enter_context(tc.tile_pool(name="w2tp", bufs=8))
            w2lp = ctx.enter_context(tc.tile_pool(name="w2lp", bufs=6))
            curp = ctx.enter_context(tc.tile_pool(name="curp", bufs=6))
            spk1p = ctx.enter_context(tc.tile_pool(name="spk1p", bufs=1))
            statep = ctx.enter_context(tc.tile_pool(name="statep", bufs=2))
            negzp = ctx.enter_context(tc.tile_pool(name="negzp", bufs=1))
            outp = ctx.enter_context(tc.tile_pool(name="outp", bufs=2))
            tpsum = ctx.enter_context(
                tc.tile_pool(name="tpsum", bufs=2, space="PSUM"))
            mpsum = ctx.enter_context(
                tc.tile_pool(name="mpsum", bufs=6, space="PSUM"))

            ident = const.tile([P, P], F32, name="ident")
            make_identity(nc, ident)

            # PE phase chaining (order-only deps): keeps fp32-mm, bf16-mm
            # and transpose phases from interleaving in the PE stream.
            pe_phases = []

            class _Ph:
                def __init__(self):
                    self.insts = []

                def add(self, bi):
                    self.insts.append(bi.ins)

            b1s = const.tile([P, HC1], F32, name="b1s")
            nc.gpsimd.dma_start(
                b1s[:], b1_d.ap().rearrange("(c p) -> p c", p=P))
            b2s = const.tile([P, HC2], F32, name="b2s")
            nc.gpsimd.dma_start(
                b2s[:], b2_d.ap().rearrange("(c p) -> p c", p=P))

            # ---------------- PE warmup (HAM ramp) --------------------------
            wub = const.tile([P, 256], BF16, name="wub")
            nc.vector.memset(wub[:], 0.0)
            wuw = const.tile([P, P], BF16, name="wuw")
            nc.vector.memset(wuw[:], 0.0)
            ph = _Ph()
            pe_phases.append(ph)
            wups = mpsum.tile([P, NB32], F32, tag="mm", name="wups")
            for i in range(34):
                ph.add(nc.tensor.matmul(wups[:, 0:256], wuw[:], wub[:],
                                        start=(i == 0), stop=(i == 33)))

            # ---------------- initial LIF state ----------------------------
            mem1_cur = statep.tile([P, HC1, 32], F32, tag="mem1",
                                   name="mem1_0")
            nc.vector.memset(mem1_cur[:], 0.0)
            mem2_cur = statep.tile([P, HC2, 32], F32, tag="mem2",
                                   name="mem2_0")
            nc.vector.memset(mem2_cur[:], 0.0)
            spk2_fin = const.tile([P, HC2, 32], F32, name="spk2_fin")

            # ---------------- outputs helper --------------------------------
            def emit_out(state, nch, out_d):
                ph = _Ph()
                pe_phases.append(ph)
                for g in range(nch // 4):
                    ps = tpsum.tile([32, 4 * P], F32, tag="tp", name="ops")
                    for j in range(4):
                        ph.add(nc.tensor.transpose(
                            ps[:, j * P:(j + 1) * P],
                            state[:, g * 4 + j, :], ident[:]))
                    sb = outp.tile([32, 4 * P], F32, tag="osb", name="osb")
                    nc.scalar.activation(sb[:], ps[:], AF.Copy)
                    nc.sync.dma_start(
                        out_d.ap()[:, g * 4 * P:(g + 1) * 4 * P], sb[:])

            # ---------------- per-block emitters ----------------------------
            def x_and_mm1(nb):
                """xT load + matmul1 for block nb -> cur1_subs"""
                ph = _Ph()
                pe_phases.append(ph)
                t0 = nb * T_NB
                xt = xtp.tile([P, KC1, 2, NB32], F16, tag="xt", name="xt")
                # block 0 is latency-critical at startup: spread its 8 chunk
                # loads over two otherwise-idle queues (weights own sync/
                # scalar); later blocks prefetch leisurely on gpsimd.
                for kc in range(KC1):
                    if nb == 0 and kc == 0:
                        # hi half first: the first matmuls' rhs
                        for j in range(2):
                            nc.gpsimd.dma_start(
                                xt[:, 0, j, :],
                                xt_d.ap()[0:P, j,
                                          t0 * 32:(t0 + T_NB) * 32])
                        continue
                    nc.gpsimd.dma_start(
                        xt[:, kc, :, :],
                        xt_d.ap()[kc * P:(kc + 1) * P, :,
                                  t0 * 32:(t0 + T_NB) * 32])

                cur1_subs = [curp.tile([P, SUBG, HC1, 32], F32, tag="cur1",
                                       bufs=3, name="cur1")
                             for _ in range(NSUBG)]
                for hq in range(HC1 // HCQ):
                    pss = [mpsum.tile([P, NB32], F32, tag="mm", name="mm1ps")
                           for _ in range(HCQ)]
                    for kc in range(KC1):
                        w1tt = w1tp.tile([P, 2, HCQ * P], F16, tag="w1t",
                                         name="w1tt")
                        dq = nc.sync if kc % 2 == 0 else nc.scalar
                        if nb == 0 and hq == 0 and kc == 0:
                            # first-needed stationaries in small pieces
                            for j in range(2):
                                for ic in range(HCQ):
                                    dq.dma_start(
                                        w1tt[:, j, ic * P:(ic + 1) * P],
                                        w1t_d.ap()[0:P, j,
                                                   ic * P:(ic + 1) * P])
                        else:
                            dq.dma_start(
                                w1tt[:],
                                w1t_d.ap()[kc * P:(kc + 1) * P, :,
                                           hq * HCQ * P:(hq + 1) * HCQ * P])
                        rhs_h = xt[:, kc, 0, :]
                        rhs_l = xt[:, kc, 1, :]
                        for i in range(HCQ):
                            # W1h*xh + W1h*xl + W1l*xh (~22-bit effective);
                            # consecutive same-stationary passes share LDW.
                            ph.add(nc.tensor.matmul(
                                pss[i][:], w1tt[:, 0, i * P:(i + 1) * P],
                                rhs_h, start=(kc == 0), stop=False))
                            ph.add(nc.tensor.matmul(
                                pss[i][:], w1tt[:, 0, i * P:(i + 1) * P],
                                rhs_l, start=False, stop=False))
                            ph.add(nc.tensor.matmul(
                                pss[i][:], w1tt[:, 1, i * P:(i + 1) * P],
                                rhs_h, start=False, stop=(kc == KC1 - 1)))
                    for s in range(NSUBG):
                        for i in range(HCQ):
                            hc = hq * HCQ + i
                            psv = pss[i].rearrange("p (t b) -> p t b", b=32)
                            nc.scalar.activation(
                                cur1_subs[s][:, :, hc, :],
                                psv[:, s * SUBG:(s + 1) * SUBG, :],
                                AF.Identity, bias=b1s[:, hc:hc + 1],
                                scale=SC1)
                return cur1_subs

            # ---------------- scan emitters ---------------------------------
            def scan1(cur1_subs):
                """LIF-1 scan (T_NB steps) -> (fp16 {0,1} spikes,
                fp8 {0,2^-8} spikes) tiles, both [(kc,t,b)]."""
                nonlocal mem1_cur
                spk1 = spk1p.tile([P, HC1, NB32], F16, tag="spk1", bufs=2,
                                  name="spk1")
                spk8 = spk1p.tile([P, HC1, NB32], F8, tag="spk8", bufs=2,
                                  name="spk8")
                for tr in range(T_NB):
                    cur_t = cur1_subs[tr // SUBG][:, tr % SUBG]
                    negz = negzp.tile([P, HC1, 32], F32, tag="negz",
                                      name="negz")
                    nc.vector.scalar_tensor_tensor(
                        negz[:], mem1_cur[:], THR, cur_t,
                        ALU.is_gt, ALU.subtract)
                    mem1_new = statep.tile([P, HC1, 32], F32, tag="mem1",
                                           name="mem1")
                    nc.vector.scalar_tensor_tensor(
                        mem1_new[:], mem1_cur[:], BETA, negz[:],
                        ALU.mult, ALU.subtract)
                    mem1_cur = mem1_new
                    # spike of step t thresholds the POST-update membrane
                    nc.vector.tensor_scalar(
                        spk1[:, :, tr * 32:(tr + 1) * 32], mem1_cur[:],
                        THR, None, ALU.is_gt)
                    nc.vector.tensor_scalar(
                        spk8[:, :, tr * 32:(tr + 1) * 32], mem1_cur[:],
                        THR, 1.0 / 256.0, ALU.is_gt, ALU.mult)
                return spk1, spk8

            def mm2(spks):
                """cur2[(t,mc,b)] = W2 @ spk1^T + b2: one fp16 hi pass +
                one fp8 DoubleRow residual pass (K=256/inst), same PSUM."""
                spk1, spk8 = spks
                ph = _Ph()
                pe_phases.append(ph)
                cur2_subs = [curp.tile([P, SUBG, HC2, 32], F32, tag="cur2",
                                       bufs=2, name="cur2")
                             for _ in range(NSUBG)]
                KP = HC1 // 2
                for mq in range(HC2 // MCQ):
                    pss = [mpsum.tile([P, NB32], F32, tag="mm",
                                      name="mm2ps")
                           for _ in range(MCQ)]
                    for kc in range(HC1):
                        wt = w2tp.tile([P, MCQ * P], F16, tag="w2t",
                                       name="w2t")
                        dq = nc.sync if kc % 2 == 0 else nc.scalar
                        dq.dma_start(
                            wt[:],
                            w2t_d.ap()[kc * P:(kc + 1) * P,
                                       mq * MCQ * P:(mq + 1) * MCQ * P])
                        rhs = spk1[:, kc, :]
                        for i in range(MCQ):
                            ph.add(nc.tensor.matmul(
                                pss[i][:], wt[:, i * P:(i + 1) * P], rhs,
                                start=(kc == 0), stop=False))
                    for q in range(KP):
                        wl = w2lp.tile([P, 2, MCQ * P], F8, tag="w2l",
                                       name="w2l")
                        nc.gpsimd.dma_start(
                            wl[:],
                            w2l_d.ap()[q * P:(q + 1) * P, :,
                                       mq * MCQ * P:(mq + 1) * MCQ * P])
                        rhs = spk8[:, 2 * q:2 * q + 2, :]
                        for i in range(MCQ):
                            ph.add(nc.tensor.matmul(
                                pss[i][:], wl[:, :, i * P:(i + 1) * P], rhs,
                                start=False, stop=(q == KP - 1),
                                perf_mode=mybir.MatmulPerfMode.DoubleRow))
                    # sub-major evac order so scan2 step 0's inputs (s=0 of
                    # every mc) complete as early as possible
                    for s in range(NSUBG):
                        for i in range(MCQ):
                            mc = mq * MCQ + i
                            psv = pss[i].rearrange("p (t b) -> p t b", b=32)
                            nc.scalar.activation(
                                cur2_subs[s][:, :, mc, :],
                                psv[:, s * SUBG:(s + 1) * SUBG, :],
                                AF.Identity, bias=b2s[:, mc:mc + 1],
                                scale=SC2)
                return cur2_subs

            def scan2(cur2_subs, nb):
                nonlocal mem2_cur
                t0 = nb * T_NB
                for tr in range(T_NB):
                    t = t0 + tr
                    cur_t = cur2_subs[tr // SUBG][:, tr % SUBG]
                    negz = negzp.tile([P, HC2, 32], F32, tag="negz",
                                      name="negz")
                    nc.vector.scalar_tensor_tensor(
                        negz[:], mem2_cur[:], THR, cur_t,
                        ALU.is_gt, ALU.subtract)
                    mem2_new = statep.tile([P, HC2, 32], F32, tag="mem2",
                                           name="mem2")
                    nc.vector.scalar_tensor_tensor(
                        mem2_new[:], mem2_cur[:], BETA, negz[:],
                        ALU.mult, ALU.subtract)
                    mem2_cur = mem2_new
                    if t == T - 1:
                        nc.vector.tensor_scalar(
                            spk2_fin[:], mem2_cur[:], THR, None, ALU.is_gt)

            # ---------------- main t-block pipeline -------------------------
            # Software pipelining, two levels:
            #  - PE stream: mm1(nb+1) is emitted BEFORE mm2(nb) so the PE
            #    fills the scan1 latency instead of stalling on spk1.
            #  - DVE stream: scan1(nb+1) is emitted BEFORE scan2(nb) so the
            #    (FIFO) vector engine runs scan1(nb+1) during mm2(nb) instead
            #    of queuing it behind scan2(nb), which can only start once
            #    mm2(nb) is nearly done. This keeps spk1(nb+1) ready the
            #    moment mm2(nb) retires -- critical for the last block, where
            #    no mm1(nb+1) exists to hide the wait.
            cur1_next = x_and_mm1(0)
            spk1_next = scan1(cur1_next)
            for nb in range(NNB):
                spk1_cur = spk1_next
                if nb + 1 < NNB:
                    cur1_next = x_and_mm1(nb + 1)
                if nb == NNB - 1:
                    emit_out(mem1_cur, HC1, mem1_d)
                cur2_subs = mm2(spk1_cur)
                if nb + 1 < NNB:
                    spk1_next = scan1(cur1_next)
                scan2(cur2_subs, nb)

            # ---------------- remaining outputs -----------------------------
            emit_out(mem2_cur, HC2, mem2_d)
            emit_out(spk2_fin, HC2, spk2_d)

            # chain consecutive PE phases: every inst of phase b ordered
            # after the last inst of phase a (order-only deps)
            for a, b in zip(pe_phases, pe_phases[1:]):
                if a.insts and b.insts:
                    for bi in b.insts:
                        add_dep_helper(bi, a.insts[-1], sync=False,
                                       reason="PE phase ordering")

    nc.compile()
    return nc


_NC_CACHE = {}


def _get_nc():
    if "full" not in _NC_CACHE:
        _NC_CACHE["full"] = build_snn()
    return _NC_CACHE["full"]


def _dekker_f16(a):
    """Split fp32 array into fp16 hi+lo terms stacked on axis 1."""
    hi = a.astype(np.float16)
    lo = (a - hi.astype(np.float32)).astype(np.float16)
    return np.ascontiguousarray(np.stack([hi, lo], axis=1))


def prep_inputs(x, W1, b1, W2, b2):
    """Host-side prep: shard x over cores (transposed to [d, (t,b)]) and
    Dekker-split x, W1, W2 into pre-scaled fp16 hi+lo pairs."""
    x = np.asarray(x, np.float32)
    W1 = np.asarray(W1, np.float32)
    b1 = np.ascontiguousarray(np.asarray(b1, np.float32))
    W2 = np.asarray(W2, np.float32)
    b2 = np.ascontiguousarray(np.asarray(b2, np.float32))
    B, T, D = x.shape

    W1Thl = _dekker_f16(W1.T * np.float32(S_W1))        # [D, 2, H1]

    # W2 -> fp16 hi + fp8 residual (scaled 2^8) in DoubleRow pair layout
    H1 = W2.shape[0]
    W2s = np.ascontiguousarray(W2.T) * np.float32(S_W2)  # [H1, H2]
    W2Th = W2s.astype(np.float16)
    r = (W2s - W2Th.astype(np.float32)) * np.float32(256.0)
    W2l8 = np.ascontiguousarray(
        r.astype(ml_dtypes.float8_e4m3)
        .reshape(H1 // 256, 2, 128, -1)
        .transpose(0, 2, 1, 3)
        .reshape(H1 // 2, 2, -1))                       # [(q p), j, H2]

    bl = B // N_CORES
    in_maps = []
    for c in range(N_CORES):
        xc = x[c * bl:(c + 1) * bl]                     # [bl, T, D]
        xT = xc.transpose(2, 1, 0).reshape(D, T * bl)   # [d, (t,b)] t-major
        xThl = _dekker_f16(xT * np.float32(S_X))        # [D, 2, (t,b)]
        in_maps.append({
            "xThl": xThl, "W1Thl": W1Thl, "b1": b1, "W2Th": W2Th,
            "W2l8": W2l8, "b2": b2,
        })
    return in_maps


def kernel(x, W1, b1, W2, b2):
    """Full-input entry point: shards B across 8 NeuronCores, returns full
    (spk2, mem1, mem2) exactly like reference()."""
    nc = _get_nc()
    in_maps = prep_inputs(x, W1, b1, W2, b2)
    res = run_bass_kernel_spmd(nc, in_maps, core_ids=list(range(N_CORES)))
    spk2 = np.concatenate([res.results[c]["spk2"] for c in range(N_CORES)], 0)
    mem1 = np.concatenate([res.results[c]["mem1"] for c in range(N_CORES)], 0)
    mem2 = np.concatenate([res.results[c]["mem2"] for c in range(N_CORES)], 0)
    return spk2, mem1, mem2

